# revision 60
# baseline (speedup 1.0000x reference)
"""Trainium2 Bass kernel for nn_Attention_11158325035119.

Reference computation (B=2, N=2048, DIM=1024, H=16, DH=64):
  LayerNorm(x) -> Q,K,V projections -> softmax(Q K^T) V (raw logits, no
  1/sqrt(d) scale) -> output projection.

Sharding over 8 NeuronCores: data-parallel on batch (2 groups of 4 cores),
tensor-parallel on heads within each group (4 heads/core, Wq/Wkv
column-sharded).  Instead of the classic Wout-row-shard + AllReduce (8MB
AllReduce per group, ~100us at the very end), each core's normalized
attention output is redistributed with a per-head AllToAll (overlapped with
the remaining heads' compute) so every core ends up with all heads for a
quarter of its batch's rows and computes a disjoint out-row-slice.  The host
then just concatenates the 8 slices.

The runtime only supports AllToAll on >4-core mesh groups, so the A2A runs
over all 8 cores: each core duplicates its 4 lane-shards into both groups'
slots, and the output projection contracts over a doubled inner dimension
(2048) against a per-core Wout whose other-group row-blocks are zero
(host-prepared).  That keeps the program SPMD (no core-id branching).

Per-core pipeline (v8: fp16 QKV path, bf16 attention path):
  1. LayerNorm (bn_stats/bn_aggr, rows-on-partitions); the first two x
     tiles are strip-loaded across the sync+scalar DMA queues
  2. PE-transpose (fp16) -> xnT [feat, seq]
  3. Q^T, K^T = Wq/k^T @ xnT, V = xnT^T @ Wv  (fp16 weights DMA'd directly
     on the gpsimd queue; 1 cyc/row matmuls with FWL weight loads)
  4. per head: S^T[k,q] = K Q^T (fp16 in, fp32 PSUM); exp on ScalarE
     (PSUM fp32 -> SBUF bf16; no max-subtraction: |logits| < ~50 so exp
     stays in range; bf16 needed for e^{+-40} range); O^T_ext = [V|1]^T @
     expS (bf16, M=65: row 64 accumulates the softmax denominators inside
     the same matmul).  32 early S/exp steps (heads 0-1, K/Q chunks 0-1)
     are emitted inside the prefix so exp overlaps the projection chains;
     a zero_gate [P,1] tile (rewritten after every LN Sqrt, used as the
     early exps' zero bias) keeps all Sqrts before all Exps in the ScalarE
     stream -- each sqrt<->exp flip would reload the ACT table (~1.3us).
  5. O^T *= 1/denom: denom row bounced through DRAM to [64,32], DVE recip,
     cast, seed partitions 0/32, one stream_shuffle broadcast, DVE mul.
     All dispatches on the sync queue (the gpsimd stream blocks at the
     Ofull-gather A2A waits); nothing touches the PE stream or PSUM.
  6. AllToAll (per head, 8 cores): heads <-> q-row-slices; h-major Ofull
     placement (head h -> col-blocks 2h, 2h+1) with host-permuted Wout
  7. out_slice = O_full^T.T @ Wout2 (bf16) -> fp32 [512, 1024], phase 0
     (heads 0-2, 48 links) hidden under the h3 A2A wait, phase 1 (head 3,
     16 links) as the only post-A2A tail

gamma/beta are applied generically (they are ones/zeros in this problem's
setup_inputs, but the kernel does not rely on that).

Measured: ~357-367us HW exec (baseline 409-414us), rel err 3.7e-3.
Engine profile at 367us: PE union-busy ~77% (the binding engine; HAM
power/activity throttling keeps it at K=4/8 or 13/16 for much of the
attention phase), ScalarE exp ~147us, DVE ~34%.
"""

import numpy as np

import concourse.bass as bass
import concourse.tile as tile
from concourse import mybir
from concourse.masks import make_identity

F32 = mybir.dt.float32
F32R = mybir.dt.float32r
BF16 = mybir.dt.bfloat16
F16 = mybir.dt.float16

EPS = 1e-5

B, N, DIM = 2, 2048, 1024
H, DH = 16, 64
N_CORES = 8
LANES = 4            # cores per batch group (head-parallel)
HL = H // LANES      # local heads per core


# ---------------------------------------------------------------------------
# Environment workarounds
# ---------------------------------------------------------------------------

def _install_drain_split():
    """walrus in this image rejects InstDrain with >1 sem wait ("Too many
    sync wait commands").  Replace the TileContext tail drain with a chain
    of drains, each waiting on a single proc's semaphore."""
    import re
    import bass_rust

    def _split_drain_and_barrier(self, tick_clock, wait_clock):
        nc = self.nc
        gc = tick_clock.global_clock
        ticks = [int(v) for v in re.findall(r"\d+", repr(gc))]
        for proc, t in [(i, t) for i, t in enumerate(ticks) if t > 0]:
            pc = bass_rust.VectorClock()
            pc.require_at_least(proc, t)
            d = nc.sync.drain()
            wait_clock.add_sem_waits(d.ins, bass_rust.ScopedClock({None: pc}))
        nc.all_engine_barrier()
        assert self.sems is not None
        popped = nc._tile_sem_poison_stack.pop()
        assert popped is self._sem_poison
        nc.clear_and_free_semaphores(list(self.sems.allocated().values()))
        nc.all_engine_barrier()

    tile.TileContext._drain_and_barrier = _split_drain_and_barrier


def _install_profile_shim():
    """Provide antenv.axon_hooks (NTFF profiling via libaxon_pjrt.so) and a
    no-op upload_artifacts (no artifact bucket in this container)."""
    import sys
    import types
    import contextlib
    import ctypes
    import os
    import concourse.bass_utils as bu

    if "antenv.axon_hooks" not in sys.modules:
        hook = None
        so_path = "/opt/axon/libaxon_pjrt.so"
        if os.path.exists(so_path):
            lib = ctypes.CDLL(so_path)
            if hasattr(lib, "axon_start_nrt_profile"):
                lib.axon_start_nrt_profile.argtypes = [
                    ctypes.POINTER(ctypes.c_int64), ctypes.c_size_t]
                lib.axon_start_nrt_profile.restype = ctypes.c_int64
                lib.axon_stop_nrt_profile.argtypes = [ctypes.c_char_p]
                lib.axon_stop_nrt_profile.restype = ctypes.c_int64

                @contextlib.contextmanager
                def _hook(output_dir, device_ids):
                    import jax
                    jax.devices()
                    if device_ids:
                        ids = (ctypes.c_int64 * len(device_ids))(*device_ids)
                        rc = lib.axon_start_nrt_profile(ids, len(device_ids))
                    else:
                        rc = lib.axon_start_nrt_profile(None, 0)
                    if rc != 0:
                        raise RuntimeError(f"axon_start_nrt_profile rc={rc}")
                    try:
                        yield
                    finally:
                        lib.axon_stop_nrt_profile(str(output_dir).encode())
                hook = _hook
        mod = types.ModuleType("antenv.axon_hooks")
        mod.get_axon_ntff_profile_hook = lambda: hook
        mod.set_axon_ntff_profile_hook = lambda h: None
        sys.modules["antenv.axon_hooks"] = mod

    bu.upload_artifacts = lambda tmpdir: f"file://{tmpdir}"


_NOPW = [0]


def split_multi_waits(nc):
    """walrus in this image rejects any engine instruction carrying more
    than one semaphore wait ("Too many sync wait commands").  Hoist extra
    waits onto InstNoOps inserted immediately before the instruction on the
    same engine — semantically identical (the waits are a conjunction and
    execute in stream order)."""
    for f in nc.m.functions:
        for blk in f.blocks:
            il = blk.instructions
            i = 0
            while i < len(il):
                inst = il[i]
                si = inst.sync_info
                if si is not None and si.on_wait is not None \
                        and len(si.on_wait) > 1:
                    waits = list(si.on_wait)
                    inst.sync_info = mybir.SyncInfo(
                        on_wait=[waits[-1]],
                        on_update=list(si.on_update or []))
                    for w in waits[:-1]:
                        _NOPW[0] += 1
                        nop = mybir.InstNoOp(name=f"nopw-{_NOPW[0]}")
                        nop.engine = inst.engine
                        nop.sync_info = mybir.SyncInfo(on_wait=[w],
                                                       on_update=[])
                        il.insert(i, nop)
                        i += 1
                i += 1
    return nc


def _install_neff_cache():
    """Disk-cache walrus NEFF compiles by bir_json content hash (a fresh
    process otherwise pays the full 10-25 min neuronxcc compile every run)."""
    import hashlib
    import os
    import shutil
    import concourse.bass_utils as bu
    import concourse.bass2jax as b2j

    cache_dir = os.environ.get(
        "BASS_NEFF_CACHE_DIR",
        os.path.join(os.path.dirname(os.path.abspath(__file__)), ".neff_cache"))
    os.makedirs(cache_dir, exist_ok=True)
    orig = bu.compile_bir_kernel

    def cached(bir_json, tmpdir, neff_name="file.neff"):
        key = hashlib.sha256(bir_json).hexdigest()[:32]
        hit = os.path.join(cache_dir, key + ".neff")
        dst = os.path.join(tmpdir, neff_name)
        if os.path.exists(hit):
            shutil.copy(hit, dst)
            return dst
        neff = orig(bir_json, tmpdir, neff_name=neff_name)
        try:
            shutil.copy(neff, hit)
        except OSError:
            pass
        return neff

    bu.compile_bir_kernel = cached
    b2j.compile_bir_kernel = cached


_install_drain_split()
_install_profile_shim()
_install_neff_cache()


# ---------------------------------------------------------------------------
# Device program
# ---------------------------------------------------------------------------

def build(nc: bass.Bass, use_f32r=True, use_a2a=True, has_bias=False):
    """Emit the per-core Tile program (SPMD: cores differ only in data).

    v3 structure (probe-driven):
      - gamma is folded into Wq/Wk/Wv on the host; beta becomes per-proj bias
        vectors applied with one extra contraction-1 matmul link per chain
        (only when beta is nonzero: has_bias).
      - LayerNorm normalize runs on GpSimd; transposes write grouped [128,512]
        PSUM tiles copied to xnT (f32r) by Vector in 512-wide slabs.
      - Attention is ScalarE-exp-bound, so the PE stream is organized to keep
        exp back-to-back: S^T steps run ahead, and the O^T accumulation work
        of the PREVIOUS half-head is emitted as 8-link sub-chain bursts
        between S steps (the PE loses ~2x throughput on every S<->O stream
        transition, so transitions are kept to ~16/head instead of 64/head).
        exp output (E) is staged in an SBUF ring (~26 tiles) bridging the
        half-head production->consumption lag.
    """
    from collections import deque

    P = 128
    S, D = N, DIM
    ST = S // P          # 16 seq tiles
    DT = D // P          # 8 feat tiles
    NQ = S // 512        # 4 q chunks
    HD = HL * DH         # 256 local head cols
    QSL = S // LANES     # 512 output rows per core
    QT = QSL // P        # 4
    GROUPS = [list(range(N_CORES))]

    # v4: the whole QKV/attention pipeline runs bf16 (within the 2e-2
    # tolerance): bf16 weights DMA directly (no stage+cast), LDWEIGHTS gets
    # FWL (fp32 loads in 2 half passes), and the S matmul moves 1024 cols
    # per instruction instead of 512.
    MMF = F16

    x_in = nc.dram_tensor("x", [S, D], F32, kind="ExternalInput").ap()
    wq_in = nc.dram_tensor("wq", [D, HD], F16, kind="ExternalInput").ap()
    wk_in = nc.dram_tensor("wk", [D, HD], F16, kind="ExternalInput").ap()
    wv_in = nc.dram_tensor("wv", [D, HD], F16, kind="ExternalInput").ap()
    if has_bias:
        bq_in = nc.dram_tensor("bq", [HD], F32, kind="ExternalInput").ap()
        bk_in = nc.dram_tensor("bk", [HD], F32, kind="ExternalInput").ap()
        bv_in = nc.dram_tensor("bv", [HD], F32, kind="ExternalInput").ap()
    if use_a2a:
        # The 8-core AllToAll delivers both groups' head blocks; "sel" (per
        # core 1.0/0.0) drives a branch-free merge picking this core's group
        # so the output projection contracts only D (not 2D of half-zeros).
        wout_in = nc.dram_tensor("wout", [D, D], BF16,
                                 kind="ExternalInput").ap()
        sel_in = nc.dram_tensor("sel", [P], F32, kind="ExternalInput").ap()
        out_dram = nc.dram_tensor("out", [QSL, D], F32,
                                  kind="ExternalOutput").ap()
        a2a_in = [nc.dram_tensor(f"a2a_in{h}", [N_CORES, DH, QSL], BF16).ap()
                  for h in range(HL)]
        a2a_out = [nc.dram_tensor(f"a2a_out{h}", [N_CORES, DH, QSL], BF16).ap()
                   for h in range(HL)]
        KTO = DT         # out-proj contraction tiles
    else:
        # no-collective fallback: emit the local partial product over the
        # core's 4 heads for ALL rows; host sums the 4 partials per batch.
        wout_in = nc.dram_tensor("woutp", [HD, D], BF16,
                                 kind="ExternalInput").ap()
        out_dram = nc.dram_tensor("out", [S, D], F32,
                                  kind="ExternalOutput").ap()
        KTO = HD // P    # 2
    den_dram = [nc.dram_tensor(f"den{h}", [DH, S // DH], F32).ap()
                for h in range(HL)]
    denb_dram = [nc.dram_tensor(f"denb{h}", [DH, S // DH], BF16).ap()
                 for h in range(HL)]

    with tile.TileContext(nc) as tc:
        with (
            tc.tile_pool(name="const", bufs=1) as const,
            tc.tile_pool(name="big", bufs=1) as big,
        ):
            # ---- small constants ----
            eps_sb = const.tile([P, 1], F32)
            nc.vector.memset(eps_sb, EPS)
            # rewritten (to 0.0) after every LN Sqrt; used as the zero bias
            # of the early Exp activations so every Sqrt precedes every Exp
            # in the ScalarE stream (a sqrt<->exp flip costs a ~1.3us ACT
            # table reload)
            zero_gate = const.tile([P, 1], F32)
            nc.vector.memset(zero_gate, 0.0)
            ident = const.tile([P, P], F16)
            make_identity(nc, ident)
            if has_bias:
                ones_row = const.tile([1, 512], MMF)
                nc.vector.memset(ones_row, 1.0)

            # ---- activations that live through attention ----
            QT_sb = big.tile([P, HD // P, S], MMF)
            KT_sb = big.tile([P, HD // P, S], MMF)
            V_sb = big.tile([P, ST, HL, DH + 1], BF16)
            nc.vector.memset(V_sb[:, :, :, DH:DH + 1], 1.0)
            # exp outputs for the early (in-prefix) attention steps of
            # heads 0/1 (t 0-7, q chunks 0-1): [128, 512] quarters
            e_early = big.tile([P, 32, 512], BF16)
            e_map = {}
            e_done = set()
            EARLY = [(h, t, c) for h in (0, 1)
                     for t in range(8) for c in (0, 1)]
            if use_a2a:
                Ofull = big.tile([P, 2 * KTO, QSL], BF16)
                Om = big.tile([P, KTO, QSL], BF16)
                sel_sb = big.tile([P, 1], F32)
                nc.sync.dma_start(out=sel_sb,
                                  in_=sel_in.rearrange("(p o) -> p o", p=P))
            else:
                obf_all = big.tile([P, KTO, S], BF16)

            # ======== prefix: LN + transpose + projections (scoped) ========
            with (
                tc.tile_pool(name="xnp", bufs=1) as xnp,
                tc.tile_pool(name="wstage", bufs=1) as wstage,
                tc.tile_pool(name="xp", bufs=2) as xp,
                tc.tile_pool(name="xnbuf", bufs=2) as xnbuf,
                tc.tile_pool(name="stats", bufs=4) as stats,
            ):

                def load_weight(name, src):
                    # bf16 weights DMA straight into SBUF; the gpsimd queue
                    # keeps them off the sync queue that feeds x tiles.
                    w = xnp.tile([P, DT, HD], F16, tag=name, name=name)
                    nc.gpsimd.dma_start(
                        out=w, in_=src.rearrange("(o p) m -> p o m", p=P))
                    return w

                wk_sb = load_weight("wk", wk_in)
                wq_sb = load_weight("wq", wq_in)
                wv_sb = load_weight("wv", wv_in)

                bias_sb = {}
                if has_bias:
                    for name, src in (("bq", bq_in), ("bk", bk_in),
                                      ("bv", bv_in)):
                        bstage = wstage.tile([1, HD], F32, tag="bstage",
                                             name=f"bstage_{name}")
                        nc.gpsimd.dma_start(out=bstage, in_=src)
                        b = xnp.tile([1, HD], MMF, tag="bias", name=name)
                        nc.vector.tensor_copy(out=b, in_=bstage)
                        bias_sb[name] = b

                with (
                    tc.tile_pool(name="tp", bufs=2, space="PSUM") as tp,
                    tc.tile_pool(name="proj", bufs=2, space="PSUM") as proj,
                    tc.tile_pool(name="vproj", bufs=2,
                                 space="PSUM") as vproj,
                    tc.tile_pool(name="searly", bufs=2,
                                 space="PSUM") as searly,
                ):
                    def ln_tile(st, xnT):
                        x_t = xp.tile([P, D], F32, tag="x",
                                      name=f"x_{st}")
                        # spread x tiles over all three dispatch queues so
                        # many DMA engines stream concurrently (a single
                        # 512KB DMA takes ~20us on one queue engine); tile
                        # 0 is split 4 ways so the LN pipeline starts fast.
                        G = 8 if st == 0 else (4 if st == 1 else 1)
                        engs = [nc.sync, nc.scalar]
                        for gi in range(G):
                            w0 = gi * (D // G)
                            engs[gi % 2].dma_start(
                                out=x_t[:, w0:w0 + D // G],
                                in_=x_in[st * P:(st + 1) * P,
                                         w0:w0 + D // G])
                        stt = stats.tile([P, 8, 6], F32, tag="stt")
                        GS = max(G, 2)
                        for gi in range(GS):
                            w0 = gi * (D // GS)
                            nc.vector.bn_stats(
                                out=stt[:, gi], in_=x_t[:, w0:w0 + D // GS])
                        mv = stats.tile([P, 2], F32, tag="mv")
                        nc.vector.bn_aggr(out=mv, in_=stt[:, 0:GS])
                        std = stats.tile([P, 1], F32, tag="std")
                        nc.scalar.activation(
                            out=std, in_=mv[:, 1:2],
                            func=mybir.ActivationFunctionType.Sqrt,
                            bias=eps_sb)
                        rstd = stats.tile([P, 1], F32, tag="rstd")
                        nc.vector.reciprocal(out=rstd, in_=std)
                        nc.vector.tensor_scalar_mul(
                            out=zero_gate, in0=std, scalar1=0.0)
                        xn_t = xnbuf.tile([P, D], F16, tag="xn",
                                          name=f"xn_{st}")
                        nc.vector.tensor_scalar(
                            out=xn_t, in0=x_t, scalar1=mv[:, 0:1],
                            scalar2=rstd,
                            op0=mybir.AluOpType.subtract,
                            op1=mybir.AluOpType.mult)
                        sti = st % 4
                        for g in range(2):
                            pt_ps = tp.tile([P, 512], F16, tag="tp")
                            for j in range(4):
                                ft = g * 4 + j
                                nc.tensor.transpose(
                                    pt_ps[:, j * P:(j + 1) * P],
                                    xn_t[:, ft * P:(ft + 1) * P], ident)
                            nc.vector.tensor_copy(
                                out=xnT[:, g * 4:(g + 1) * 4,
                                        sti * P:(sti + 1) * P],
                                in_=pt_ps.rearrange(
                                    "p (a b) -> p a b", a=4))

                    def qk_chain(w_sb, bname, dst, pt, nch, xnT):
                        ps = proj.tile([P, 512], F32, tag="proj")
                        for kt in range(DT):
                            nc.tensor.matmul(
                                ps, w_sb[:, kt, pt * P:(pt + 1) * P],
                                xnT[:, kt, :],
                                start=(kt == 0),
                                stop=(kt == DT - 1 and not has_bias))
                        if has_bias:
                            nc.tensor.matmul(
                                ps, bias_sb[bname][0:1, pt * P:(pt + 1) * P],
                                ones_row, start=False, stop=True)
                        nc.vector.tensor_copy(
                            out=dst[:, pt, nch * 512:(nch + 1) * 512],
                            in_=ps)

                    def early_step(i):
                        # S + exp for (h, t, c) that only needs K/Q chunks
                        # 0-1: overlaps ScalarE exp with the remaining
                        # prefix PE work.  The S matmul is unfloored (the
                        # scheduler places it mid-prefix on the PE), but the
                        # exp is floored past the whole modeled prefix so
                        # every LN Sqrt PRECEDES every Exp in the ScalarE
                        # stream -- otherwise the scheduler weaves them and
                        # each sqrt<->exp flip costs a ~1.3us ACT table
                        # load.
                        h, t, c = EARLY[i]
                        kb = (h * DH) % P
                        kpt = (h * DH) // P
                        s_ps = searly.tile([P, 512], F32, tag="se",
                                           name=f"se_{h}_{t}_{c}")
                        nc.tensor.matmul(
                            s_ps,
                            KT_sb[kb:kb + DH, kpt, t * P:(t + 1) * P],
                            QT_sb[kb:kb + DH, kpt, c * 512:(c + 1) * 512],
                            start=True, stop=True)
                        ei = e_early[:, i, :]
                        nc.scalar.activation(
                            out=ei, in_=s_ps,
                            func=mybir.ActivationFunctionType.Exp,
                            bias=zero_gate)
                        e_map[(h, t, c)] = ei
                        e_done.add((h, t, c))

                    for nch in range(NQ):
                        xnT = xnp.tile([P, DT, 512], MMF, tag="xnT",
                                       name=f"xnT_{nch}")
                        for sti in range(4):
                            ln_tile(nch * 4 + sti, xnT)
                        if nch == NQ - 1:
                            # all 16 LN Sqrt activations are now emitted, so
                            # the exp stream can start without thrashing the
                            # ACT table set; these overlap the last chunk's
                            # projection chains on the PE.
                            for i in range(len(EARLY)):
                                early_step(i)
                        for pt in range(HD // P):
                            qk_chain(wk_sb, "bk", KT_sb, pt, nch, xnT)
                        for pt in range(HD // P):
                            qk_chain(wq_sb, "bq", QT_sb, pt, nch, xnT)
                        for sti in range(4):
                            st = nch * 4 + sti
                            ps = vproj.tile([P, HD], F32, tag="vproj")
                            for kt in range(DT):
                                nc.tensor.matmul(
                                    ps, xnT[:, kt, sti * P:(sti + 1) * P],
                                    wv_sb[:, kt, :],
                                    start=(kt == 0),
                                    stop=(kt == DT - 1 and not has_bias))
                            if has_bias:
                                nc.tensor.matmul(
                                    ps, ones_row[0:1, 0:P], bias_sb["bv"],
                                    start=False, stop=True)
                            nc.vector.tensor_copy(
                                out=V_sb[:, st, :, 0:DH],
                                in_=ps.rearrange("p (h d) -> p h d", h=HL))


            # ======== attention (exp-bound, S-ahead / O-subchain bursts) ====
            with (
                tc.tile_pool(name="late", bufs=1) as late,
                tc.tile_pool(name="expp", bufs=26) as expp,
                tc.tile_pool(name="obfp", bufs=2) as obfp,
                tc.tile_pool(name="bcast", bufs=2) as bcast,
                tc.tile_pool(name="outp", bufs=2) as outp,
            ):
                # out-proj weights: DMA overlaps attention (allocated here
                # so the slot reuses SBUF freed by the prefix pools)
                wout_sb = late.tile([P, KTO, D], BF16, name="wout_sb")
                nc.sync.dma_start(out=wout_sb,
                                  in_=wout_in.rearrange("(o p) m -> p o m",
                                                        p=P))
                # two rotating reciprocal-broadcast buffers, zero-filled
                # once up front (stream_shuffle reads the whole tile)
                rec_bufs = [late.tile([DH, S], BF16, name=f"rec_buf{i}")
                            for i in range(2)]
                for rb in rec_bufs:
                    nc.vector.memset(rb, 0.0)

                def finish_head(h, o_ps):
                    # stage O_ext to SBUF at once: frees the 4 o_ps PSUM
                    # banks; normalize + AllToAll overlap the next head.
                    o_sb = bcast.tile([DH + 1, S], F32, tag="osum",
                                      name=f"o_sb_{h}")
                    for c in range(NQ):
                        nc.vector.tensor_copy(
                            out=o_sb[:, c * 512:(c + 1) * 512],
                            in_=o_ps[c])
                    # denominators live on partition 64 as a [1, S] row.
                    # Direct SBUF->SBUF partition-scatter DMA to a [64, 32]
                    # layout, recip there (single-partition recip is ~16us
                    # on DVE), cast bf16, gather back into partitions 0 and
                    # 32, then one DVE stream_shuffle broadcasts within
                    # each 32-partition quadrant.  All dispatches ride the
                    # sync queue: the gpsimd stream stalls at the
                    # Ofull-gather A2A waits, and the PE stream is
                    # untouched.
                    nc.sync.dma_start(out=den_dram[h].rearrange(
                        "j m -> (j m)"), in_=o_sb[DH:DH + 1, :])
                    dn = bcast.tile([DH, S // DH], F32, tag="dn",
                                    name=f"dn_{h}")
                    nc.sync.dma_start(out=dn, in_=den_dram[h])
                    nc.vector.reciprocal(out=dn, in_=dn)
                    dnr = bcast.tile([DH, S // DH], BF16, tag="dnr",
                                     name=f"dnr_{h}")
                    nc.vector.tensor_copy(out=dnr, in_=dn)
                    nc.sync.dma_start(out=denb_dram[h].rearrange(
                        "j m -> (j m)"), in_=dnr)
                    rec_b = rec_bufs[h % 2]
                    rbv = rec_b.rearrange("(a b) q -> a b q", a=2)
                    for a in range(2):
                        nc.sync.dma_start(
                            out=rbv[a:a + 1, 0:1, :],
                            in_=denb_dram[h].rearrange("j m -> (j m)"))
                    nc.vector.stream_shuffle(out=rec_b, in_=rec_b,
                                             mask=[0] * 32)
                    if use_a2a:
                        obf_h = obfp.tile([DH, S], BF16, tag="obf")
                    else:
                        inner = h * DH
                        obf_h = obf_all[inner % P:inner % P + DH,
                                        inner // P, :]
                    nc.vector.tensor_mul(
                        out=obf_h, in0=o_sb[0:DH, :], in1=rec_b)
                    if use_a2a:
                        # lane shard j duplicated into both groups' slots;
                        # 4 DMAs across two queues so the 512KB write is
                        # parallel across DMA engines.
                        for half in range(2):
                            for piece in range(2):
                                j0 = piece * 2
                                nc.sync.dma_start(
                                    out=a2a_in[h][half * LANES + j0:
                                                  half * LANES + j0 + 2]
                                    .rearrange("j p q -> p j q"),
                                    in_=obf_h[:, j0 * QSL:(j0 + 2) * QSL]
                                    .rearrange("p (j q) -> p j q", j=2))
                        nc.gpsimd.collective_compute(
                            "AllToAll", mybir.AluOpType.bypass,
                            replica_groups=GROUPS,
                            ins=[a2a_in[h][:]], outs=[a2a_out[h][:]])
                        # h-major gather: head h's blocks land at Ofull
                        # col-blocks g*8 + 2h + {0,1}; lanes (0,1)/(2,3)
                        # fill partitions 0-127 of each block.
                        for g in range(2):
                            nc.gpsimd.dma_start(
                                out=Ofull[:, g * KTO + 2 * h:
                                          g * KTO + 2 * h + 2, :],
                                in_=a2a_out[h][g * LANES:(g + 1) * LANES]
                                .rearrange("(lb lt) d q -> (lt d) lb q",
                                           lt=2))

                with (
                    tc.tile_pool(name="spsum", bufs=2, space="PSUM") as spsum,
                    tc.tile_pool(name="opsum", bufs=NQ, space="PSUM") as opsum,
                ):
                    pend = deque()
                    o_ps_by_head = {}
                    head_windows = {}

                    QL = ST // 2     # t-links per drained sub-chain

                    def check_window(h, t, c):
                        tlo = (t // QL) * QL
                        if all((h, t2, c) in e_done
                               for t2 in range(tlo, tlo + QL)):
                            pend.append((h, c, tlo))

                    def emit_subchain():
                        h2, c, tlo = pend.popleft()
                        if h2 not in o_ps_by_head:
                            o_ps_by_head[h2] = [
                                opsum.tile([DH + 1, 512], F32, tag="o",
                                           name=f"o_ps_{h2}_{cc}")
                                for cc in range(NQ)]
                        o_ps = o_ps_by_head[h2]
                        for t2 in range(tlo, tlo + QL):
                            nc.tensor.matmul(
                                o_ps[c], V_sb[:, t2, h2, :],
                                e_map[(h2, t2, c)],
                                start=(t2 == 0), stop=(t2 == ST - 1))
                        head_windows[h2] = head_windows.get(h2, 0) + 1
                        if head_windows[h2] == 2 * NQ:
                            finish_head(h2, o_ps_by_head.pop(h2))

                    # The TileScheduler reorders by modeled readiness and
                    # would round-robin S and O matmuls (each S<->O stream
                    # switch costs ~2x on the PE).  Modeled-time floors pin
                    # each S-step and each O-sub-chain into its own slot;
                    # they gate only the scheduler's simulated clock, no
                    # real waits are emitted.
                    ATT_MS = 1.0
                    STEP_MS = 0.003

                    def s_step(h, t, half, step):
                        kb = (h * DH) % P
                        kpt = (h * DH) // P
                        with tc.tile_wait_until(ATT_MS + step * STEP_MS):
                            s_ps = spsum.tile(
                                [P, S // 2], F32, tag="s",
                                name=f"s_ps_{h}_{t}_{half}")
                            for cc in range(NQ // 2):
                                c = half * (NQ // 2) + cc
                                nc.tensor.matmul(
                                    s_ps[:, cc * 512:(cc + 1) * 512],
                                    KT_sb[kb:kb + DH, kpt,
                                          t * P:(t + 1) * P],
                                    QT_sb[kb:kb + DH, kpt,
                                          c * 512:(c + 1) * 512],
                                    start=True, stop=True)
                            e_t = expp.tile([P, S // 2], BF16,
                                            tag="e",
                                            name=f"e_t_{h}_{t}_{half}")
                            nc.scalar.activation(
                                out=e_t, in_=s_ps,
                                func=mybir.ActivationFunctionType.Exp)
                            for cc in range(NQ // 2):
                                c = half * (NQ // 2) + cc
                                e_map[(h, t, c)] = \
                                    e_t[:, cc * 512:(cc + 1) * 512]
                                e_done.add((h, t, c))
                        for cc in range(NQ // 2):
                            check_window(h, t, half * (NQ // 2) + cc)

                    # steps not already covered by the early (in-prefix)
                    # emission: heads 0/1 are missing half 1 of t0-7 and
                    # all of t8-15; heads 2/3 everything.
                    MAIN = []
                    for h in (0, 1):
                        MAIN += [(h, t, 1) for t in range(QL)]
                        MAIN += [(h, t, 0) for t in range(QL, ST)]
                        MAIN += [(h, t, 1) for t in range(QL, ST)]
                    for h in (2, 3):
                        for t in range(ST):
                            MAIN += [(h, t, 0), (h, t, 1)]
                    # windows fully produced by the early steps
                    for h in (0, 1):
                        for c in (0, 1):
                            check_window(h, 0, c)

                    step = 0
                    for (h, t, half) in MAIN:
                        s_step(h, t, half, step)
                        if step % 2 == 1 and pend:
                            with tc.tile_wait_until(
                                    ATT_MS + step * STEP_MS + STEP_MS / 2):
                                emit_subchain()
                        step += 1
                    while pend:
                        with tc.tile_wait_until(
                                ATT_MS + step * STEP_MS + STEP_MS / 2):
                            emit_subchain()
                        step += 1

                # ---- merge + output projection ----
                # h-major Ofull layout: head h owns col-blocks {2h, 2h+1}
                # (group 0) and {2h+8, 2h+9} (group 1); Om[j] = sel ?
                # Ofull[j] : Ofull[j+KTO].  Phase 0 (heads 0-2, 48 links)
                # runs hidden under the h3 A2A wait; phase 1 (head 3, 16
                # links) is the only post-A2A tail.
                if use_a2a:
                    OM_MS = 2.0
                    with tc.tile_pool(name="oproj", bufs=8,
                                      space="PSUM") as oproj:
                        pss = {}
                        for phase, heads in enumerate(((0, 1, 2), (3,))):
                            ks = [j for hh in heads
                                  for j in (2 * hh, 2 * hh + 1)]
                            with tc.tile_wait_until(OM_MS + phase * 0.05):
                                for j in ks:
                                    mtmp = outp.tile([P, QSL], F32,
                                                     tag="mtmp")
                                    nc.vector.tensor_sub(
                                        out=mtmp,
                                        in0=Ofull[:, j, :],
                                        in1=Ofull[:, j + KTO, :])
                                    nc.vector.scalar_tensor_tensor(
                                        out=Om[:, j, :], in0=mtmp,
                                        scalar=sel_sb,
                                        in1=Ofull[:, j + KTO, :],
                                        op0=mybir.AluOpType.mult,
                                        op1=mybir.AluOpType.add)
                            with tc.tile_wait_until(OM_MS + 0.01
                                                    + phase * 0.05):
                                for qt in range(QT):
                                    for nch in range(D // 512):
                                        if phase == 0:
                                            pss[(qt, nch)] = oproj.tile(
                                                [P, 512], F32, tag="op",
                                                name=f"op_{qt}_{nch}")
                                        ps = pss[(qt, nch)]
                                        for i2, kt in enumerate(ks):
                                            nc.tensor.matmul(
                                                ps,
                                                Om[:, kt,
                                                   qt * P:(qt + 1) * P],
                                                wout_sb[:, kt,
                                                        nch * 512:
                                                        (nch + 1) * 512],
                                                start=(phase == 0 and
                                                       i2 == 0),
                                                stop=(phase == 1 and
                                                      i2 == len(ks) - 1))
                        for qt in range(QT):
                            ot = outp.tile([P, D], F32, tag="ot")
                            for nch in range(D // 512):
                                nc.vector.tensor_copy(
                                    out=ot[:, nch * 512:(nch + 1) * 512],
                                    in_=pss[(qt, nch)])
                            # alternate dispatch queues so the final 2MB
                            # write streams on more DMA engines (ScalarE is
                            # idle after the last exp)
                            eng = nc.sync if qt % 2 == 0 else nc.scalar
                            eng.dma_start(
                                out=out_dram[qt * P:(qt + 1) * P, :],
                                in_=ot)
                else:
                    with tc.tile_pool(name="oproj", bufs=4,
                                      space="PSUM") as oproj:
                        for qt in range(ST):
                            ot = outp.tile([P, D], F32, tag="ot")
                            for nch in range(D // 512):
                                ps = oproj.tile([P, 512], F32, tag="op")
                                for kt in range(KTO):
                                    nc.tensor.matmul(
                                        ps,
                                        obf_all[:, kt, qt * P:(qt + 1) * P],
                                        wout_sb[:, kt,
                                                nch * 512:(nch + 1) * 512],
                                        start=(kt == 0),
                                        stop=(kt == KTO - 1))
                                nc.vector.tensor_copy(
                                    out=ot[:, nch * 512:(nch + 1) * 512],
                                    in_=ps)
                            nc.sync.dma_start(
                                out=out_dram[qt * P:(qt + 1) * P, :],
                                in_=ot)

    return nc


# ---------------------------------------------------------------------------
# Host entry point
# ---------------------------------------------------------------------------

_CACHE = {}
USE_A2A = True
USE_F32R = True


def _get_program(has_bias=False):
    key = (USE_A2A, USE_F32R, has_bias)
    if key not in _CACHE:
        nc = bass.Bass("TRN2", target_bir_lowering=False, debug=False,
                       num_devices=N_CORES)
        build(nc, use_f32r=USE_F32R, use_a2a=USE_A2A, has_bias=has_bias)
        split_multi_waits(nc)
        _CACHE[key] = nc
    return _CACHE[key]


def _shard_inputs(x, gamma, beta, Wq, Wkv, Wout):
    import ml_dtypes
    x = np.asarray(x, dtype=np.float32)
    gamma = np.asarray(gamma, dtype=np.float32)
    beta = np.asarray(beta, dtype=np.float32)
    Wq = np.asarray(Wq, dtype=np.float32)
    Wkv = np.asarray(Wkv, dtype=np.float32)
    Wk, Wv = Wkv[:, :H * DH], Wkv[:, H * DH:]
    # LayerNorm affine folded into the projections:
    #   (xn*gamma + beta) @ W = xn @ (gamma[:,None]*W) + beta @ W
    Wq_g = gamma[:, None] * Wq
    Wk_g = gamma[:, None] * Wk
    Wv_g = gamma[:, None] * Wv
    has_bias = bool(np.any(beta))
    if has_bias:
        bq = beta @ Wq_g
        bk = beta @ Wk_g
        bv = beta @ Wv_g
    wout_f = np.asarray(Wout, np.float32)
    wout_bf = np.ascontiguousarray(wout_f).astype(ml_dtypes.bfloat16)
    # h-major inner layout for the A2A path: Om row h*256 + lane*64 + d
    # holds global head (lane*4 + h) -> permute Wout rows to match.
    wout_perm = np.ascontiguousarray(
        wout_f.reshape(LANES, HL, DH, DIM).transpose(1, 0, 2, 3)
        .reshape(DIM, DIM)).astype(ml_dtypes.bfloat16)
    in_maps = []
    for core in range(N_CORES):
        b = core // LANES
        lane = core % LANES
        cs = slice(lane * HL * DH, (lane + 1) * HL * DH)
        m = {
            "x": np.ascontiguousarray(x[b]),
            "wq": np.ascontiguousarray(Wq_g[:, cs]).astype(np.float16),
            "wk": np.ascontiguousarray(Wk_g[:, cs]).astype(np.float16),
            "wv": np.ascontiguousarray(Wv_g[:, cs]).astype(np.float16),
        }
        if has_bias:
            m["bq"] = np.ascontiguousarray(bq[cs])
            m["bk"] = np.ascontiguousarray(bk[cs])
            m["bv"] = np.ascontiguousarray(bv[cs])
        if USE_A2A:
            m["wout"] = wout_perm
            m["sel"] = np.full((128,), 1.0 if b == 0 else 0.0,
                               dtype=np.float32)
        else:
            m["woutp"] = np.ascontiguousarray(wout_bf[cs.start:cs.stop])
        in_maps.append(m)
    return in_maps


def _unshard_output(results):
    out = np.empty((B, N, DIM), dtype=np.float32)
    if USE_A2A:
        qsl = N // LANES
        for core in range(N_CORES):
            b = core // LANES
            lane = core % LANES
            out[b, lane * qsl:(lane + 1) * qsl, :] = results[core]["out"]
    else:
        for b in range(B):
            acc = results[b * LANES]["out"].astype(np.float32).copy()
            for lane in range(1, LANES):
                acc += results[b * LANES + lane]["out"]
            out[b] = acc
    return out


def kernel(x, gamma, beta, Wq, Wkv, Wout, trace=False):
    from concourse.bass_utils import run_bass_kernel_spmd
    has_bias = bool(np.any(np.asarray(beta, dtype=np.float32)))
    nc = _get_program(has_bias)
    in_maps = _shard_inputs(x, gamma, beta, Wq, Wkv, Wout)
    res = run_bass_kernel_spmd(nc, in_maps, list(range(N_CORES)), trace=trace)
    out = _unshard_output(res.results)
    if trace:
        kernel.last_exec_time_ns = res.exec_time_ns
        kernel.last_result = res
    return out



# revision 61
# speedup vs baseline: 1.0188x; 1.0188x over previous
"""Trainium2 Bass kernel for nn_Attention_11158325035119.

Reference computation (B=2, N=2048, DIM=1024, H=16, DH=64):
  LayerNorm(x) -> Q,K,V projections -> softmax(Q K^T) V (raw logits, no
  1/sqrt(d) scale) -> output projection.

Sharding over 8 NeuronCores: data-parallel on batch (2 groups of 4 cores),
tensor-parallel on heads within each group (4 heads/core, Wq/Wkv
column-sharded).  Instead of the classic Wout-row-shard + AllReduce (8MB
AllReduce per group, ~100us at the very end), each core's normalized
attention output is redistributed with a per-head AllToAll (overlapped with
the remaining heads' compute) so every core ends up with all heads for a
quarter of its batch's rows and computes a disjoint out-row-slice.  The host
then just concatenates the 8 slices.

The runtime only supports AllToAll on >4-core mesh groups, so the A2A runs
over all 8 cores: each core duplicates its 4 lane-shards into both groups'
slots, and the output projection contracts over a doubled inner dimension
(2048) against a per-core Wout whose other-group row-blocks are zero
(host-prepared).  That keeps the program SPMD (no core-id branching).

Per-core pipeline (v8: fp16 QKV path, bf16 attention path):
  1. LayerNorm (bn_stats/bn_aggr, rows-on-partitions); the first two x
     tiles are strip-loaded across the sync+scalar DMA queues
  2. PE-transpose (fp16) -> xnT [feat, seq]
  3. Q^T, K^T = Wq/k^T @ xnT, V = xnT^T @ Wv  (fp16 weights DMA'd directly
     on the gpsimd queue; 1 cyc/row matmuls with FWL weight loads)
  4. per head: S^T[k,q] = K Q^T (fp16 in, fp32 PSUM); exp on ScalarE
     (PSUM fp32 -> SBUF bf16; no max-subtraction: |logits| < ~50 so exp
     stays in range; bf16 needed for e^{+-40} range); O^T_ext = [V|1]^T @
     expS (bf16, M=65: row 64 accumulates the softmax denominators inside
     the same matmul).  32 early S/exp steps (heads 0-1, K/Q chunks 0-1)
     are emitted inside the prefix so exp overlaps the projection chains;
     a zero_gate [P,1] tile (rewritten after every LN Sqrt, used as the
     early exps' zero bias) keeps all Sqrts before all Exps in the ScalarE
     stream -- each sqrt<->exp flip would reload the ACT table (~1.3us).
  5. O^T *= 1/denom: denom row bounced through DRAM to [64,32], DVE recip,
     cast, seed partitions 0/32, one stream_shuffle broadcast, DVE mul.
     All dispatches on the sync queue (the gpsimd stream blocks at the
     Ofull-gather A2A waits); nothing touches the PE stream or PSUM.
  6. AllToAll (per head, 8 cores): heads <-> q-row-slices; h-major Ofull
     placement (head h -> col-blocks 2h, 2h+1) with host-permuted Wout
  7. out_slice = O_full^T.T @ Wout2 (bf16) -> fp32 [512, 1024], phase 0
     (heads 0-2, 48 links) hidden under the h3 A2A wait, phase 1 (head 3,
     16 links) as the only post-A2A tail

gamma/beta are applied generically (they are ones/zeros in this problem's
setup_inputs, but the kernel does not rely on that).

Measured: ~357-367us HW exec (baseline 409-414us), rel err 3.7e-3.
Engine profile at 367us: PE union-busy ~77% (the binding engine; HAM
power/activity throttling keeps it at K=4/8 or 13/16 for much of the
attention phase), ScalarE exp ~147us, DVE ~34%.
"""

import numpy as np

import concourse.bass as bass
import concourse.tile as tile
from concourse import mybir
from concourse.masks import make_identity

F32 = mybir.dt.float32
F32R = mybir.dt.float32r
BF16 = mybir.dt.bfloat16
F16 = mybir.dt.float16

EPS = 1e-5

B, N, DIM = 2, 2048, 1024
H, DH = 16, 64
N_CORES = 8
LANES = 4            # cores per batch group (head-parallel)
HL = H // LANES      # local heads per core


# ---------------------------------------------------------------------------
# Environment workarounds
# ---------------------------------------------------------------------------

def _install_drain_split():
    """walrus in this image rejects InstDrain with >1 sem wait ("Too many
    sync wait commands").  Replace the TileContext tail drain with a chain
    of drains, each waiting on a single proc's semaphore."""
    import re
    import bass_rust

    def _split_drain_and_barrier(self, tick_clock, wait_clock):
        nc = self.nc
        gc = tick_clock.global_clock
        ticks = [int(v) for v in re.findall(r"\d+", repr(gc))]
        for proc, t in [(i, t) for i, t in enumerate(ticks) if t > 0]:
            pc = bass_rust.VectorClock()
            pc.require_at_least(proc, t)
            d = nc.sync.drain()
            wait_clock.add_sem_waits(d.ins, bass_rust.ScopedClock({None: pc}))
        nc.all_engine_barrier()
        assert self.sems is not None
        popped = nc._tile_sem_poison_stack.pop()
        assert popped is self._sem_poison
        nc.clear_and_free_semaphores(list(self.sems.allocated().values()))
        nc.all_engine_barrier()

    tile.TileContext._drain_and_barrier = _split_drain_and_barrier


def _install_profile_shim():
    """Provide antenv.axon_hooks (NTFF profiling via libaxon_pjrt.so) and a
    no-op upload_artifacts (no artifact bucket in this container)."""
    import sys
    import types
    import contextlib
    import ctypes
    import os
    import concourse.bass_utils as bu

    if "antenv.axon_hooks" not in sys.modules:
        hook = None
        so_path = "/opt/axon/libaxon_pjrt.so"
        if os.path.exists(so_path):
            lib = ctypes.CDLL(so_path)
            if hasattr(lib, "axon_start_nrt_profile"):
                lib.axon_start_nrt_profile.argtypes = [
                    ctypes.POINTER(ctypes.c_int64), ctypes.c_size_t]
                lib.axon_start_nrt_profile.restype = ctypes.c_int64
                lib.axon_stop_nrt_profile.argtypes = [ctypes.c_char_p]
                lib.axon_stop_nrt_profile.restype = ctypes.c_int64

                @contextlib.contextmanager
                def _hook(output_dir, device_ids):
                    import jax
                    jax.devices()
                    if device_ids:
                        ids = (ctypes.c_int64 * len(device_ids))(*device_ids)
                        rc = lib.axon_start_nrt_profile(ids, len(device_ids))
                    else:
                        rc = lib.axon_start_nrt_profile(None, 0)
                    if rc != 0:
                        raise RuntimeError(f"axon_start_nrt_profile rc={rc}")
                    try:
                        yield
                    finally:
                        lib.axon_stop_nrt_profile(str(output_dir).encode())
                hook = _hook
        mod = types.ModuleType("antenv.axon_hooks")
        mod.get_axon_ntff_profile_hook = lambda: hook
        mod.set_axon_ntff_profile_hook = lambda h: None
        sys.modules["antenv.axon_hooks"] = mod

    bu.upload_artifacts = lambda tmpdir: f"file://{tmpdir}"


_NOPW = [0]


def split_multi_waits(nc):
    """walrus in this image rejects any engine instruction carrying more
    than one semaphore wait ("Too many sync wait commands").  Hoist extra
    waits onto InstNoOps inserted immediately before the instruction on the
    same engine — semantically identical (the waits are a conjunction and
    execute in stream order)."""
    for f in nc.m.functions:
        for blk in f.blocks:
            il = blk.instructions
            i = 0
            while i < len(il):
                inst = il[i]
                si = inst.sync_info
                if si is not None and si.on_wait is not None \
                        and len(si.on_wait) > 1:
                    waits = list(si.on_wait)
                    inst.sync_info = mybir.SyncInfo(
                        on_wait=[waits[-1]],
                        on_update=list(si.on_update or []))
                    for w in waits[:-1]:
                        _NOPW[0] += 1
                        nop = mybir.InstNoOp(name=f"nopw-{_NOPW[0]}")
                        nop.engine = inst.engine
                        nop.sync_info = mybir.SyncInfo(on_wait=[w],
                                                       on_update=[])
                        il.insert(i, nop)
                        i += 1
                i += 1
    return nc


def _install_neff_cache():
    """Disk-cache walrus NEFF compiles by bir_json content hash (a fresh
    process otherwise pays the full 10-25 min neuronxcc compile every run)."""
    import hashlib
    import os
    import shutil
    import concourse.bass_utils as bu
    import concourse.bass2jax as b2j

    cache_dir = os.environ.get(
        "BASS_NEFF_CACHE_DIR",
        os.path.join(os.path.dirname(os.path.abspath(__file__)), ".neff_cache"))
    os.makedirs(cache_dir, exist_ok=True)
    orig = bu.compile_bir_kernel

    def cached(bir_json, tmpdir, neff_name="file.neff"):
        key = hashlib.sha256(bir_json).hexdigest()[:32]
        hit = os.path.join(cache_dir, key + ".neff")
        dst = os.path.join(tmpdir, neff_name)
        if os.path.exists(hit):
            shutil.copy(hit, dst)
            return dst
        neff = orig(bir_json, tmpdir, neff_name=neff_name)
        try:
            shutil.copy(neff, hit)
        except OSError:
            pass
        return neff

    bu.compile_bir_kernel = cached
    b2j.compile_bir_kernel = cached


_install_drain_split()
_install_profile_shim()
_install_neff_cache()


# ---------------------------------------------------------------------------
# Device program
# ---------------------------------------------------------------------------

def build(nc: bass.Bass, use_f32r=True, use_a2a=True, has_bias=False):
    """Emit the per-core Tile program (SPMD: cores differ only in data).

    v3 structure (probe-driven):
      - gamma is folded into Wq/Wk/Wv on the host; beta becomes per-proj bias
        vectors applied with one extra contraction-1 matmul link per chain
        (only when beta is nonzero: has_bias).
      - LayerNorm normalize runs on GpSimd; transposes write grouped [128,512]
        PSUM tiles copied to xnT (f32r) by Vector in 512-wide slabs.
      - Attention is ScalarE-exp-bound, so the PE stream is organized to keep
        exp back-to-back: S^T steps run ahead, and the O^T accumulation work
        of the PREVIOUS half-head is emitted as 8-link sub-chain bursts
        between S steps (the PE loses ~2x throughput on every S<->O stream
        transition, so transitions are kept to ~16/head instead of 64/head).
        exp output (E) is staged in an SBUF ring (~26 tiles) bridging the
        half-head production->consumption lag.
    """
    from collections import deque

    P = 128
    S, D = N, DIM
    ST = S // P          # 16 seq tiles
    DT = D // P          # 8 feat tiles
    NQ = S // 512        # 4 q chunks
    HD = HL * DH         # 256 local head cols
    QSL = S // LANES     # 512 output rows per core
    QT = QSL // P        # 4
    GROUPS = [list(range(N_CORES))]

    # v4: the whole QKV/attention pipeline runs bf16 (within the 2e-2
    # tolerance): bf16 weights DMA directly (no stage+cast), LDWEIGHTS gets
    # FWL (fp32 loads in 2 half passes), and the S matmul moves 1024 cols
    # per instruction instead of 512.
    MMF = F16

    x_in = nc.dram_tensor("x", [S, D], F32, kind="ExternalInput").ap()
    wq_in = nc.dram_tensor("wq", [D, HD], F16, kind="ExternalInput").ap()
    wk_in = nc.dram_tensor("wk", [D, HD], F16, kind="ExternalInput").ap()
    wv_in = nc.dram_tensor("wv", [D, HD], F16, kind="ExternalInput").ap()
    if has_bias:
        bq_in = nc.dram_tensor("bq", [HD], F32, kind="ExternalInput").ap()
        bk_in = nc.dram_tensor("bk", [HD], F32, kind="ExternalInput").ap()
        bv_in = nc.dram_tensor("bv", [HD], F32, kind="ExternalInput").ap()
    if use_a2a:
        # The 8-core AllToAll delivers both groups' head blocks; "sel" (per
        # core 1.0/0.0) drives a branch-free merge picking this core's group
        # so the output projection contracts only D (not 2D of half-zeros).
        wout_in = nc.dram_tensor("wout", [D, D], BF16,
                                 kind="ExternalInput").ap()
        sel_in = nc.dram_tensor("sel", [P], F32, kind="ExternalInput").ap()
        out_dram = nc.dram_tensor("out", [QSL, D], F32,
                                  kind="ExternalOutput").ap()
        a2a_in = [nc.dram_tensor(f"a2a_in{h}", [N_CORES, DH, QSL], BF16).ap()
                  for h in range(HL)]
        a2a_out = [nc.dram_tensor(f"a2a_out{h}", [N_CORES, DH, QSL], BF16).ap()
                   for h in range(HL)]
        KTO = DT         # out-proj contraction tiles
    else:
        # no-collective fallback: emit the local partial product over the
        # core's 4 heads for ALL rows; host sums the 4 partials per batch.
        wout_in = nc.dram_tensor("woutp", [HD, D], BF16,
                                 kind="ExternalInput").ap()
        out_dram = nc.dram_tensor("out", [S, D], F32,
                                  kind="ExternalOutput").ap()
        KTO = HD // P    # 2
    den_dram = [nc.dram_tensor(f"den{h}", [DH, S // DH], F32).ap()
                for h in range(HL)]
    denb_dram = [nc.dram_tensor(f"denb{h}", [DH, S // DH], BF16).ap()
                 for h in range(HL)]

    with tile.TileContext(nc) as tc:
        with (
            tc.tile_pool(name="const", bufs=1) as const,
            tc.tile_pool(name="big", bufs=1) as big,
        ):
            # ---- small constants ----
            eps_sb = const.tile([P, 1], F32)
            nc.vector.memset(eps_sb, EPS)
            # rewritten (to 0.0) after every LN Sqrt; used as the zero bias
            # of the early Exp activations so every Sqrt precedes every Exp
            # in the ScalarE stream (a sqrt<->exp flip costs a ~1.3us ACT
            # table reload)
            zero_gate = const.tile([P, 1], F32)
            nc.vector.memset(zero_gate, 0.0)
            ident = const.tile([P, P], F16)
            make_identity(nc, ident)
            if has_bias:
                ones_row = const.tile([1, 512], MMF)
                nc.vector.memset(ones_row, 1.0)

            # ---- activations that live through attention ----
            QT_sb = big.tile([P, HD // P, S], MMF)
            KT_sb = big.tile([P, HD // P, S], MMF)
            V_sb = big.tile([P, ST, HL, DH + 1], BF16)
            nc.vector.memset(V_sb[:, :, :, DH:DH + 1], 1.0)
            # exp outputs for the early (in-prefix) attention steps of
            # heads 0/1 (t 0-7, q chunks 0-1): [128, 512] quarters
            e_early = big.tile([P, 32, 512], BF16)
            e_map = {}
            e_done = set()
            EARLY = [(h, t, c) for h in (0, 1)
                     for t in range(8) for c in (0, 1)]
            if use_a2a:
                Ofull = big.tile([P, 2 * KTO, QSL], BF16)
                Om = big.tile([P, KTO, QSL], BF16)
                sel_sb = big.tile([P, 1], F32)
                nc.sync.dma_start(out=sel_sb,
                                  in_=sel_in.rearrange("(p o) -> p o", p=P))
            else:
                obf_all = big.tile([P, KTO, S], BF16)

            # ======== prefix: LN + transpose + projections (scoped) ========
            with (
                tc.tile_pool(name="xnp", bufs=1) as xnp,
                tc.tile_pool(name="wstage", bufs=1) as wstage,
                tc.tile_pool(name="xp", bufs=2) as xp,
                tc.tile_pool(name="xnbuf", bufs=2) as xnbuf,
                tc.tile_pool(name="stats", bufs=4) as stats,
            ):

                def load_weight(name, src):
                    # bf16 weights DMA straight into SBUF; the gpsimd queue
                    # keeps them off the sync queue that feeds x tiles.
                    w = xnp.tile([P, DT, HD], F16, tag=name, name=name)
                    nc.gpsimd.dma_start(
                        out=w, in_=src.rearrange("(o p) m -> p o m", p=P))
                    return w

                wk_sb = load_weight("wk", wk_in)
                wq_sb = load_weight("wq", wq_in)
                wv_sb = load_weight("wv", wv_in)

                bias_sb = {}
                if has_bias:
                    for name, src in (("bq", bq_in), ("bk", bk_in),
                                      ("bv", bv_in)):
                        bstage = wstage.tile([1, HD], F32, tag="bstage",
                                             name=f"bstage_{name}")
                        nc.gpsimd.dma_start(out=bstage, in_=src)
                        b = xnp.tile([1, HD], MMF, tag="bias", name=name)
                        nc.vector.tensor_copy(out=b, in_=bstage)
                        bias_sb[name] = b

                with (
                    tc.tile_pool(name="tp", bufs=2, space="PSUM") as tp,
                    tc.tile_pool(name="proj", bufs=2, space="PSUM") as proj,
                    tc.tile_pool(name="vproj", bufs=2,
                                 space="PSUM") as vproj,
                    tc.tile_pool(name="searly", bufs=2,
                                 space="PSUM") as searly,
                ):
                    def ln_tile(st, xnT):
                        x_t = xp.tile([P, D], F32, tag="x",
                                      name=f"x_{st}")
                        # spread x tiles over all three dispatch queues so
                        # many DMA engines stream concurrently (a single
                        # 512KB DMA takes ~20us on one queue engine); tile
                        # 0 is split 4 ways so the LN pipeline starts fast.
                        G = 4 if st < 2 else 1
                        engs = [nc.sync, nc.scalar]
                        for gi in range(G):
                            w0 = gi * (D // G)
                            engs[gi % 2].dma_start(
                                out=x_t[:, w0:w0 + D // G],
                                in_=x_in[st * P:(st + 1) * P,
                                         w0:w0 + D // G])
                        stt = stats.tile([P, 4, 6], F32, tag="stt")
                        GS = max(G, 2)
                        for gi in range(GS):
                            w0 = gi * (D // GS)
                            nc.vector.bn_stats(
                                out=stt[:, gi], in_=x_t[:, w0:w0 + D // GS])
                        mv = stats.tile([P, 2], F32, tag="mv")
                        nc.vector.bn_aggr(out=mv, in_=stt[:, 0:GS])
                        std = stats.tile([P, 1], F32, tag="std")
                        nc.scalar.activation(
                            out=std, in_=mv[:, 1:2],
                            func=mybir.ActivationFunctionType.Sqrt,
                            bias=eps_sb)
                        rstd = stats.tile([P, 1], F32, tag="rstd")
                        nc.vector.reciprocal(out=rstd, in_=std)
                        nc.vector.tensor_scalar_mul(
                            out=zero_gate, in0=std, scalar1=0.0)
                        xn_t = xnbuf.tile([P, D], F16, tag="xn",
                                          name=f"xn_{st}")
                        nc.vector.tensor_scalar(
                            out=xn_t, in0=x_t, scalar1=mv[:, 0:1],
                            scalar2=rstd,
                            op0=mybir.AluOpType.subtract,
                            op1=mybir.AluOpType.mult)
                        sti = st % 4
                        for g in range(2):
                            pt_ps = tp.tile([P, 512], F16, tag="tp")
                            for j in range(4):
                                ft = g * 4 + j
                                nc.tensor.transpose(
                                    pt_ps[:, j * P:(j + 1) * P],
                                    xn_t[:, ft * P:(ft + 1) * P], ident)
                            nc.vector.tensor_copy(
                                out=xnT[:, g * 4:(g + 1) * 4,
                                        sti * P:(sti + 1) * P],
                                in_=pt_ps.rearrange(
                                    "p (a b) -> p a b", a=4))

                    def qk_chain(w_sb, bname, dst, pt, nch, xnT):
                        ps = proj.tile([P, 512], F32, tag="proj")
                        for kt in range(DT):
                            nc.tensor.matmul(
                                ps, w_sb[:, kt, pt * P:(pt + 1) * P],
                                xnT[:, kt, :],
                                start=(kt == 0),
                                stop=(kt == DT - 1 and not has_bias))
                        if has_bias:
                            nc.tensor.matmul(
                                ps, bias_sb[bname][0:1, pt * P:(pt + 1) * P],
                                ones_row, start=False, stop=True)
                        nc.vector.tensor_copy(
                            out=dst[:, pt, nch * 512:(nch + 1) * 512],
                            in_=ps)

                    def early_step(i):
                        # S + exp for (h, t, c) that only needs K/Q chunks
                        # 0-1: overlaps ScalarE exp with the remaining
                        # prefix PE work.  The S matmul is unfloored (the
                        # scheduler places it mid-prefix on the PE), but the
                        # exp is floored past the whole modeled prefix so
                        # every LN Sqrt PRECEDES every Exp in the ScalarE
                        # stream -- otherwise the scheduler weaves them and
                        # each sqrt<->exp flip costs a ~1.3us ACT table
                        # load.
                        h, t, c = EARLY[i]
                        kb = (h * DH) % P
                        kpt = (h * DH) // P
                        s_ps = searly.tile([P, 512], F32, tag="se",
                                           name=f"se_{h}_{t}_{c}")
                        nc.tensor.matmul(
                            s_ps,
                            KT_sb[kb:kb + DH, kpt, t * P:(t + 1) * P],
                            QT_sb[kb:kb + DH, kpt, c * 512:(c + 1) * 512],
                            start=True, stop=True)
                        ei = e_early[:, i, :]
                        nc.scalar.activation(
                            out=ei, in_=s_ps,
                            func=mybir.ActivationFunctionType.Exp,
                            bias=zero_gate)
                        e_map[(h, t, c)] = ei
                        e_done.add((h, t, c))

                    for nch in range(NQ):
                        xnT = xnp.tile([P, DT, 512], MMF, tag="xnT",
                                       name=f"xnT_{nch}")
                        for sti in range(4):
                            ln_tile(nch * 4 + sti, xnT)
                        if nch == NQ - 1:
                            # all 16 LN Sqrt activations are now emitted, so
                            # the exp stream can start without thrashing the
                            # ACT table set; these overlap the last chunk's
                            # projection chains on the PE.
                            for i in range(len(EARLY)):
                                early_step(i)
                        for pt in range(HD // P):
                            qk_chain(wk_sb, "bk", KT_sb, pt, nch, xnT)
                        for pt in range(HD // P):
                            qk_chain(wq_sb, "bq", QT_sb, pt, nch, xnT)
                        for sti in range(4):
                            st = nch * 4 + sti
                            ps = vproj.tile([P, HD], F32, tag="vproj")
                            for kt in range(DT):
                                nc.tensor.matmul(
                                    ps, xnT[:, kt, sti * P:(sti + 1) * P],
                                    wv_sb[:, kt, :],
                                    start=(kt == 0),
                                    stop=(kt == DT - 1 and not has_bias))
                            if has_bias:
                                nc.tensor.matmul(
                                    ps, ones_row[0:1, 0:P], bias_sb["bv"],
                                    start=False, stop=True)
                            nc.vector.tensor_copy(
                                out=V_sb[:, st, :, 0:DH],
                                in_=ps.rearrange("p (h d) -> p h d", h=HL))


            # ======== attention (exp-bound, S-ahead / O-subchain bursts) ====
            with (
                tc.tile_pool(name="late", bufs=1) as late,
                tc.tile_pool(name="expp", bufs=26) as expp,
                tc.tile_pool(name="obfp", bufs=2) as obfp,
                tc.tile_pool(name="bcast", bufs=2) as bcast,
                tc.tile_pool(name="outp", bufs=2) as outp,
            ):
                # out-proj weights: DMA overlaps attention (allocated here
                # so the slot reuses SBUF freed by the prefix pools)
                wout_sb = late.tile([P, KTO, D], BF16, name="wout_sb")
                nc.sync.dma_start(out=wout_sb,
                                  in_=wout_in.rearrange("(o p) m -> p o m",
                                                        p=P))
                # two rotating reciprocal-broadcast buffers, zero-filled
                # once up front (stream_shuffle reads the whole tile)
                rec_bufs = [late.tile([DH, S], BF16, name=f"rec_buf{i}")
                            for i in range(2)]
                for rb in rec_bufs:
                    nc.vector.memset(rb, 0.0)

                def finish_head(h, o_ps):
                    # stage O_ext to SBUF at once: frees the 4 o_ps PSUM
                    # banks; normalize + AllToAll overlap the next head.
                    o_sb = bcast.tile([DH + 1, S], F32, tag="osum",
                                      name=f"o_sb_{h}")
                    for c in range(NQ):
                        nc.vector.tensor_copy(
                            out=o_sb[:, c * 512:(c + 1) * 512],
                            in_=o_ps[c])
                    # denominators live on partition 64 as a [1, S] row.
                    # Direct SBUF->SBUF partition-scatter DMA to a [64, 32]
                    # layout, recip there (single-partition recip is ~16us
                    # on DVE), cast bf16, gather back into partitions 0 and
                    # 32, then one DVE stream_shuffle broadcasts within
                    # each 32-partition quadrant.  All dispatches ride the
                    # sync queue: the gpsimd stream stalls at the
                    # Ofull-gather A2A waits, and the PE stream is
                    # untouched.
                    nc.sync.dma_start(out=den_dram[h].rearrange(
                        "j m -> (j m)"), in_=o_sb[DH:DH + 1, :])
                    dn = bcast.tile([DH, S // DH], F32, tag="dn",
                                    name=f"dn_{h}")
                    nc.sync.dma_start(out=dn, in_=den_dram[h])
                    nc.vector.reciprocal(out=dn, in_=dn)
                    dnr = bcast.tile([DH, S // DH], BF16, tag="dnr",
                                     name=f"dnr_{h}")
                    nc.vector.tensor_copy(out=dnr, in_=dn)
                    nc.sync.dma_start(out=denb_dram[h].rearrange(
                        "j m -> (j m)"), in_=dnr)
                    rec_b = rec_bufs[h % 2]
                    rbv = rec_b.rearrange("(a b) q -> a b q", a=2)
                    for a in range(2):
                        nc.sync.dma_start(
                            out=rbv[a:a + 1, 0:1, :],
                            in_=denb_dram[h].rearrange("j m -> (j m)"))
                    nc.vector.stream_shuffle(out=rec_b, in_=rec_b,
                                             mask=[0] * 32)
                    if use_a2a:
                        obf_h = obfp.tile([DH, S], BF16, tag="obf")
                    else:
                        inner = h * DH
                        obf_h = obf_all[inner % P:inner % P + DH,
                                        inner // P, :]
                    nc.vector.tensor_mul(
                        out=obf_h, in0=o_sb[0:DH, :], in1=rec_b)
                    if use_a2a:
                        # lane shard j duplicated into both groups' slots;
                        # 4 DMAs across two queues so the 512KB write is
                        # parallel across DMA engines.
                        for half in range(2):
                            for piece in range(2):
                                j0 = piece * 2
                                nc.sync.dma_start(
                                    out=a2a_in[h][half * LANES + j0:
                                                  half * LANES + j0 + 2]
                                    .rearrange("j p q -> p j q"),
                                    in_=obf_h[:, j0 * QSL:(j0 + 2) * QSL]
                                    .rearrange("p (j q) -> p j q", j=2))
                        nc.gpsimd.collective_compute(
                            "AllToAll", mybir.AluOpType.bypass,
                            replica_groups=GROUPS,
                            ins=[a2a_in[h][:]], outs=[a2a_out[h][:]])
                        # h-major gather: head h's blocks land at Ofull
                        # col-blocks g*8 + 2h + {0,1}; lanes (0,1)/(2,3)
                        # fill partitions 0-127 of each block.
                        for g in range(2):
                            nc.gpsimd.dma_start(
                                out=Ofull[:, g * KTO + 2 * h:
                                          g * KTO + 2 * h + 2, :],
                                in_=a2a_out[h][g * LANES:(g + 1) * LANES]
                                .rearrange("(lb lt) d q -> (lt d) lb q",
                                           lt=2))

                with (
                    tc.tile_pool(name="spsum", bufs=2, space="PSUM") as spsum,
                    tc.tile_pool(name="opsum", bufs=NQ, space="PSUM") as opsum,
                ):
                    pend = deque()
                    o_ps_by_head = {}
                    head_windows = {}

                    QL = ST // 2     # t-links per drained sub-chain

                    def check_window(h, t, c):
                        tlo = (t // QL) * QL
                        if all((h, t2, c) in e_done
                               for t2 in range(tlo, tlo + QL)):
                            pend.append((h, c, tlo))

                    def emit_subchain():
                        h2, c, tlo = pend.popleft()
                        if h2 not in o_ps_by_head:
                            o_ps_by_head[h2] = [
                                opsum.tile([DH + 1, 512], F32, tag="o",
                                           name=f"o_ps_{h2}_{cc}")
                                for cc in range(NQ)]
                        o_ps = o_ps_by_head[h2]
                        for t2 in range(tlo, tlo + QL):
                            nc.tensor.matmul(
                                o_ps[c], V_sb[:, t2, h2, :],
                                e_map[(h2, t2, c)],
                                start=(t2 == 0), stop=(t2 == ST - 1))
                        head_windows[h2] = head_windows.get(h2, 0) + 1
                        if head_windows[h2] == 2 * NQ:
                            finish_head(h2, o_ps_by_head.pop(h2))

                    # The TileScheduler reorders by modeled readiness and
                    # would round-robin S and O matmuls (each S<->O stream
                    # switch costs ~2x on the PE).  Modeled-time floors pin
                    # each S-step and each O-sub-chain into its own slot;
                    # they gate only the scheduler's simulated clock, no
                    # real waits are emitted.
                    ATT_MS = 1.0
                    STEP_MS = 0.003

                    def s_step(h, t, half, step):
                        kb = (h * DH) % P
                        kpt = (h * DH) // P
                        with tc.tile_wait_until(ATT_MS + step * STEP_MS):
                            s_ps = spsum.tile(
                                [P, S // 2], F32, tag="s",
                                name=f"s_ps_{h}_{t}_{half}")
                            for cc in range(NQ // 2):
                                c = half * (NQ // 2) + cc
                                nc.tensor.matmul(
                                    s_ps[:, cc * 512:(cc + 1) * 512],
                                    KT_sb[kb:kb + DH, kpt,
                                          t * P:(t + 1) * P],
                                    QT_sb[kb:kb + DH, kpt,
                                          c * 512:(c + 1) * 512],
                                    start=True, stop=True)
                            e_t = expp.tile([P, S // 2], BF16,
                                            tag="e",
                                            name=f"e_t_{h}_{t}_{half}")
                            nc.scalar.activation(
                                out=e_t, in_=s_ps,
                                func=mybir.ActivationFunctionType.Exp)
                            for cc in range(NQ // 2):
                                c = half * (NQ // 2) + cc
                                e_map[(h, t, c)] = \
                                    e_t[:, cc * 512:(cc + 1) * 512]
                                e_done.add((h, t, c))
                        for cc in range(NQ // 2):
                            check_window(h, t, half * (NQ // 2) + cc)

                    # steps not already covered by the early (in-prefix)
                    # emission: heads 0/1 are missing half 1 of t0-7 and
                    # all of t8-15; heads 2/3 everything.
                    MAIN = []
                    for h in (0, 1):
                        MAIN += [(h, t, 1) for t in range(QL)]
                        MAIN += [(h, t, 0) for t in range(QL, ST)]
                        MAIN += [(h, t, 1) for t in range(QL, ST)]
                    for h in (2, 3):
                        for t in range(ST):
                            MAIN += [(h, t, 0), (h, t, 1)]
                    # windows fully produced by the early steps
                    for h in (0, 1):
                        for c in (0, 1):
                            check_window(h, 0, c)

                    step = 0
                    for (h, t, half) in MAIN:
                        s_step(h, t, half, step)
                        if step % 2 == 1 and pend:
                            with tc.tile_wait_until(
                                    ATT_MS + step * STEP_MS + STEP_MS / 2):
                                emit_subchain()
                        step += 1
                    while pend:
                        with tc.tile_wait_until(
                                ATT_MS + step * STEP_MS + STEP_MS / 2):
                            emit_subchain()
                        step += 1

                # ---- merge + output projection ----
                # h-major Ofull layout: head h owns col-blocks {2h, 2h+1}
                # (group 0) and {2h+8, 2h+9} (group 1); Om[j] = sel ?
                # Ofull[j] : Ofull[j+KTO].  Phase 0 (heads 0-2, 48 links)
                # runs hidden under the h3 A2A wait; phase 1 (head 3, 16
                # links) is the only post-A2A tail.
                if use_a2a:
                    OM_MS = 2.0
                    with tc.tile_pool(name="oproj", bufs=8,
                                      space="PSUM") as oproj:
                        pss = {}
                        for phase, heads in enumerate(((0, 1, 2), (3,))):
                            ks = [j for hh in heads
                                  for j in (2 * hh, 2 * hh + 1)]
                            with tc.tile_wait_until(OM_MS + phase * 0.05):
                                for j in ks:
                                    mtmp = outp.tile([P, QSL], F32,
                                                     tag="mtmp")
                                    nc.vector.tensor_sub(
                                        out=mtmp,
                                        in0=Ofull[:, j, :],
                                        in1=Ofull[:, j + KTO, :])
                                    nc.vector.scalar_tensor_tensor(
                                        out=Om[:, j, :], in0=mtmp,
                                        scalar=sel_sb,
                                        in1=Ofull[:, j + KTO, :],
                                        op0=mybir.AluOpType.mult,
                                        op1=mybir.AluOpType.add)
                            with tc.tile_wait_until(OM_MS + 0.01
                                                    + phase * 0.05):
                                for qt in range(QT):
                                    for nch in range(D // 512):
                                        if phase == 0:
                                            pss[(qt, nch)] = oproj.tile(
                                                [P, 512], F32, tag="op",
                                                name=f"op_{qt}_{nch}")
                                        ps = pss[(qt, nch)]
                                        for i2, kt in enumerate(ks):
                                            nc.tensor.matmul(
                                                ps,
                                                Om[:, kt,
                                                   qt * P:(qt + 1) * P],
                                                wout_sb[:, kt,
                                                        nch * 512:
                                                        (nch + 1) * 512],
                                                start=(phase == 0 and
                                                       i2 == 0),
                                                stop=(phase == 1 and
                                                      i2 == len(ks) - 1))
                        for qt in range(QT):
                            ot = outp.tile([P, D], F32, tag="ot")
                            for nch in range(D // 512):
                                nc.vector.tensor_copy(
                                    out=ot[:, nch * 512:(nch + 1) * 512],
                                    in_=pss[(qt, nch)])
                            # alternate dispatch queues so the final 2MB
                            # write streams on more DMA engines (ScalarE is
                            # idle after the last exp)
                            eng = nc.sync if qt % 2 == 0 else nc.scalar
                            eng.dma_start(
                                out=out_dram[qt * P:(qt + 1) * P, :],
                                in_=ot)
                else:
                    with tc.tile_pool(name="oproj", bufs=4,
                                      space="PSUM") as oproj:
                        for qt in range(ST):
                            ot = outp.tile([P, D], F32, tag="ot")
                            for nch in range(D // 512):
                                ps = oproj.tile([P, 512], F32, tag="op")
                                for kt in range(KTO):
                                    nc.tensor.matmul(
                                        ps,
                                        obf_all[:, kt, qt * P:(qt + 1) * P],
                                        wout_sb[:, kt,
                                                nch * 512:(nch + 1) * 512],
                                        start=(kt == 0),
                                        stop=(kt == KTO - 1))
                                nc.vector.tensor_copy(
                                    out=ot[:, nch * 512:(nch + 1) * 512],
                                    in_=ps)
                            nc.sync.dma_start(
                                out=out_dram[qt * P:(qt + 1) * P, :],
                                in_=ot)

    return nc


# ---------------------------------------------------------------------------
# Host entry point
# ---------------------------------------------------------------------------

_CACHE = {}
USE_A2A = True
USE_F32R = True


def _get_program(has_bias=False):
    key = (USE_A2A, USE_F32R, has_bias)
    if key not in _CACHE:
        nc = bass.Bass("TRN2", target_bir_lowering=False, debug=False,
                       num_devices=N_CORES)
        build(nc, use_f32r=USE_F32R, use_a2a=USE_A2A, has_bias=has_bias)
        split_multi_waits(nc)
        _CACHE[key] = nc
    return _CACHE[key]


def _shard_inputs(x, gamma, beta, Wq, Wkv, Wout):
    import ml_dtypes
    x = np.asarray(x, dtype=np.float32)
    gamma = np.asarray(gamma, dtype=np.float32)
    beta = np.asarray(beta, dtype=np.float32)
    Wq = np.asarray(Wq, dtype=np.float32)
    Wkv = np.asarray(Wkv, dtype=np.float32)
    Wk, Wv = Wkv[:, :H * DH], Wkv[:, H * DH:]
    # LayerNorm affine folded into the projections:
    #   (xn*gamma + beta) @ W = xn @ (gamma[:,None]*W) + beta @ W
    Wq_g = gamma[:, None] * Wq
    Wk_g = gamma[:, None] * Wk
    Wv_g = gamma[:, None] * Wv
    has_bias = bool(np.any(beta))
    if has_bias:
        bq = beta @ Wq_g
        bk = beta @ Wk_g
        bv = beta @ Wv_g
    wout_f = np.asarray(Wout, np.float32)
    wout_bf = np.ascontiguousarray(wout_f).astype(ml_dtypes.bfloat16)
    # h-major inner layout for the A2A path: Om row h*256 + lane*64 + d
    # holds global head (lane*4 + h) -> permute Wout rows to match.
    wout_perm = np.ascontiguousarray(
        wout_f.reshape(LANES, HL, DH, DIM).transpose(1, 0, 2, 3)
        .reshape(DIM, DIM)).astype(ml_dtypes.bfloat16)
    in_maps = []
    for core in range(N_CORES):
        b = core // LANES
        lane = core % LANES
        cs = slice(lane * HL * DH, (lane + 1) * HL * DH)
        m = {
            "x": np.ascontiguousarray(x[b]),
            "wq": np.ascontiguousarray(Wq_g[:, cs]).astype(np.float16),
            "wk": np.ascontiguousarray(Wk_g[:, cs]).astype(np.float16),
            "wv": np.ascontiguousarray(Wv_g[:, cs]).astype(np.float16),
        }
        if has_bias:
            m["bq"] = np.ascontiguousarray(bq[cs])
            m["bk"] = np.ascontiguousarray(bk[cs])
            m["bv"] = np.ascontiguousarray(bv[cs])
        if USE_A2A:
            m["wout"] = wout_perm
            m["sel"] = np.full((128,), 1.0 if b == 0 else 0.0,
                               dtype=np.float32)
        else:
            m["woutp"] = np.ascontiguousarray(wout_bf[cs.start:cs.stop])
        in_maps.append(m)
    return in_maps


def _unshard_output(results):
    out = np.empty((B, N, DIM), dtype=np.float32)
    if USE_A2A:
        qsl = N // LANES
        for core in range(N_CORES):
            b = core // LANES
            lane = core % LANES
            out[b, lane * qsl:(lane + 1) * qsl, :] = results[core]["out"]
    else:
        for b in range(B):
            acc = results[b * LANES]["out"].astype(np.float32).copy()
            for lane in range(1, LANES):
                acc += results[b * LANES + lane]["out"]
            out[b] = acc
    return out


def kernel(x, gamma, beta, Wq, Wkv, Wout, trace=False):
    from concourse.bass_utils import run_bass_kernel_spmd
    has_bias = bool(np.any(np.asarray(beta, dtype=np.float32)))
    nc = _get_program(has_bias)
    in_maps = _shard_inputs(x, gamma, beta, Wq, Wkv, Wout)
    res = run_bass_kernel_spmd(nc, in_maps, list(range(N_CORES)), trace=trace)
    out = _unshard_output(res.results)
    if trace:
        kernel.last_exec_time_ns = res.exec_time_ns
        kernel.last_result = res
    return out



# revision 62
# speedup vs baseline: 1.0514x; 1.0320x over previous
"""Trainium2 Bass kernel for nn_Attention_11158325035119.

Reference computation (B=2, N=2048, DIM=1024, H=16, DH=64):
  LayerNorm(x) -> Q,K,V projections -> softmax(Q K^T) V (raw logits, no
  1/sqrt(d) scale) -> output projection.

Sharding over 8 NeuronCores: data-parallel on batch (2 groups of 4 cores),
tensor-parallel on heads within each group (4 heads/core, Wq/Wkv
column-sharded).  Instead of the classic Wout-row-shard + AllReduce (8MB
AllReduce per group, ~100us at the very end), each core's normalized
attention output is redistributed with a per-head AllToAll (overlapped with
the remaining heads' compute) so every core ends up with all heads for a
quarter of its batch's rows and computes a disjoint out-row-slice.  The host
then just concatenates the 8 slices.

The runtime only supports AllToAll on >4-core mesh groups, so the A2A runs
over all 8 cores: each core duplicates its 4 lane-shards into both groups'
slots, and the output projection contracts over a doubled inner dimension
(2048) against a per-core Wout whose other-group row-blocks are zero
(host-prepared).  That keeps the program SPMD (no core-id branching).

Per-core pipeline (v8: fp16 QKV path, bf16 attention path):
  1. LayerNorm (bn_stats/bn_aggr, rows-on-partitions); the first two x
     tiles are strip-loaded across the sync+scalar DMA queues
  2. PE-transpose (fp16) -> xnT [feat, seq]
  3. Q^T, K^T = Wq/k^T @ xnT, V = xnT^T @ Wv  (fp16 weights DMA'd directly
     on the gpsimd queue; 1 cyc/row matmuls with FWL weight loads)
  4. per head: S^T[k,q] = K Q^T (fp16 in, fp32 PSUM); exp on ScalarE
     (PSUM fp32 -> SBUF bf16; no max-subtraction: |logits| < ~50 so exp
     stays in range; bf16 needed for e^{+-40} range); O^T_ext = [V|1]^T @
     expS (bf16, M=65: row 64 accumulates the softmax denominators inside
     the same matmul).  32 early S/exp steps (heads 0-1, K/Q chunks 0-1)
     are emitted inside the prefix so exp overlaps the projection chains;
     a zero_gate [P,1] tile (rewritten after every LN Sqrt, used as the
     early exps' zero bias) keeps all Sqrts before all Exps in the ScalarE
     stream -- each sqrt<->exp flip would reload the ACT table (~1.3us).
  5. O^T *= 1/denom: denom row bounced through DRAM to [64,32], DVE recip,
     cast, seed partitions 0/32, one stream_shuffle broadcast, DVE mul.
     All dispatches on the sync queue (the gpsimd stream blocks at the
     Ofull-gather A2A waits); nothing touches the PE stream or PSUM.
  6. AllToAll (per head, 8 cores): heads <-> q-row-slices; h-major Ofull
     placement (head h -> col-blocks 2h, 2h+1) with host-permuted Wout
  7. out_slice = O_full^T.T @ Wout2 (bf16) -> fp32 [512, 1024], phase 0
     (heads 0-2, 48 links) hidden under the h3 A2A wait, phase 1 (head 3,
     16 links) as the only post-A2A tail

gamma/beta are applied generically (they are ones/zeros in this problem's
setup_inputs, but the kernel does not rely on that).

Measured: ~357-367us HW exec (baseline 409-414us), rel err 3.7e-3.
Engine profile at 367us: PE union-busy ~77% (the binding engine; HAM
power/activity throttling keeps it at K=4/8 or 13/16 for much of the
attention phase), ScalarE exp ~147us, DVE ~34%.
"""

import numpy as np

import concourse.bass as bass
import concourse.tile as tile
from concourse import mybir
from concourse.masks import make_identity

F32 = mybir.dt.float32
F32R = mybir.dt.float32r
BF16 = mybir.dt.bfloat16
F16 = mybir.dt.float16

EPS = 1e-5

B, N, DIM = 2, 2048, 1024
H, DH = 16, 64
N_CORES = 8
LANES = 4            # cores per batch group (head-parallel)
HL = H // LANES      # local heads per core


# ---------------------------------------------------------------------------
# Environment workarounds
# ---------------------------------------------------------------------------

def _install_drain_split():
    """walrus in this image rejects InstDrain with >1 sem wait ("Too many
    sync wait commands").  Replace the TileContext tail drain with a chain
    of drains, each waiting on a single proc's semaphore."""
    import re
    import bass_rust

    def _split_drain_and_barrier(self, tick_clock, wait_clock):
        nc = self.nc
        gc = tick_clock.global_clock
        ticks = [int(v) for v in re.findall(r"\d+", repr(gc))]
        for proc, t in [(i, t) for i, t in enumerate(ticks) if t > 0]:
            pc = bass_rust.VectorClock()
            pc.require_at_least(proc, t)
            d = nc.sync.drain()
            wait_clock.add_sem_waits(d.ins, bass_rust.ScopedClock({None: pc}))
        nc.all_engine_barrier()
        assert self.sems is not None
        popped = nc._tile_sem_poison_stack.pop()
        assert popped is self._sem_poison
        nc.clear_and_free_semaphores(list(self.sems.allocated().values()))
        nc.all_engine_barrier()

    tile.TileContext._drain_and_barrier = _split_drain_and_barrier


def _install_profile_shim():
    """Provide antenv.axon_hooks (NTFF profiling via libaxon_pjrt.so) and a
    no-op upload_artifacts (no artifact bucket in this container)."""
    import sys
    import types
    import contextlib
    import ctypes
    import os
    import concourse.bass_utils as bu

    if "antenv.axon_hooks" not in sys.modules:
        hook = None
        so_path = "/opt/axon/libaxon_pjrt.so"
        if os.path.exists(so_path):
            lib = ctypes.CDLL(so_path)
            if hasattr(lib, "axon_start_nrt_profile"):
                lib.axon_start_nrt_profile.argtypes = [
                    ctypes.POINTER(ctypes.c_int64), ctypes.c_size_t]
                lib.axon_start_nrt_profile.restype = ctypes.c_int64
                lib.axon_stop_nrt_profile.argtypes = [ctypes.c_char_p]
                lib.axon_stop_nrt_profile.restype = ctypes.c_int64

                @contextlib.contextmanager
                def _hook(output_dir, device_ids):
                    import jax
                    jax.devices()
                    if device_ids:
                        ids = (ctypes.c_int64 * len(device_ids))(*device_ids)
                        rc = lib.axon_start_nrt_profile(ids, len(device_ids))
                    else:
                        rc = lib.axon_start_nrt_profile(None, 0)
                    if rc != 0:
                        raise RuntimeError(f"axon_start_nrt_profile rc={rc}")
                    try:
                        yield
                    finally:
                        lib.axon_stop_nrt_profile(str(output_dir).encode())
                hook = _hook
        mod = types.ModuleType("antenv.axon_hooks")
        mod.get_axon_ntff_profile_hook = lambda: hook
        mod.set_axon_ntff_profile_hook = lambda h: None
        sys.modules["antenv.axon_hooks"] = mod

    bu.upload_artifacts = lambda tmpdir: f"file://{tmpdir}"


_NOPW = [0]


def split_multi_waits(nc):
    """walrus in this image rejects any engine instruction carrying more
    than one semaphore wait ("Too many sync wait commands").  Hoist extra
    waits onto InstNoOps inserted immediately before the instruction on the
    same engine — semantically identical (the waits are a conjunction and
    execute in stream order)."""
    for f in nc.m.functions:
        for blk in f.blocks:
            il = blk.instructions
            i = 0
            while i < len(il):
                inst = il[i]
                si = inst.sync_info
                if si is not None and si.on_wait is not None \
                        and len(si.on_wait) > 1:
                    waits = list(si.on_wait)
                    inst.sync_info = mybir.SyncInfo(
                        on_wait=[waits[-1]],
                        on_update=list(si.on_update or []))
                    for w in waits[:-1]:
                        _NOPW[0] += 1
                        nop = mybir.InstNoOp(name=f"nopw-{_NOPW[0]}")
                        nop.engine = inst.engine
                        nop.sync_info = mybir.SyncInfo(on_wait=[w],
                                                       on_update=[])
                        il.insert(i, nop)
                        i += 1
                i += 1
    return nc


def _install_neff_cache():
    """Disk-cache walrus NEFF compiles by bir_json content hash (a fresh
    process otherwise pays the full 10-25 min neuronxcc compile every run)."""
    import hashlib
    import os
    import shutil
    import concourse.bass_utils as bu
    import concourse.bass2jax as b2j

    cache_dir = os.environ.get(
        "BASS_NEFF_CACHE_DIR",
        os.path.join(os.path.dirname(os.path.abspath(__file__)), ".neff_cache"))
    os.makedirs(cache_dir, exist_ok=True)
    orig = bu.compile_bir_kernel

    def cached(bir_json, tmpdir, neff_name="file.neff"):
        key = hashlib.sha256(bir_json).hexdigest()[:32]
        hit = os.path.join(cache_dir, key + ".neff")
        dst = os.path.join(tmpdir, neff_name)
        if os.path.exists(hit):
            shutil.copy(hit, dst)
            return dst
        neff = orig(bir_json, tmpdir, neff_name=neff_name)
        try:
            shutil.copy(neff, hit)
        except OSError:
            pass
        return neff

    bu.compile_bir_kernel = cached
    b2j.compile_bir_kernel = cached


_install_drain_split()
_install_profile_shim()
_install_neff_cache()


# ---------------------------------------------------------------------------
# Device program
# ---------------------------------------------------------------------------

def build(nc: bass.Bass, use_f32r=True, use_a2a=True, has_bias=False):
    """Emit the per-core Tile program (SPMD: cores differ only in data).

    v3 structure (probe-driven):
      - gamma is folded into Wq/Wk/Wv on the host; beta becomes per-proj bias
        vectors applied with one extra contraction-1 matmul link per chain
        (only when beta is nonzero: has_bias).
      - LayerNorm normalize runs on GpSimd; transposes write grouped [128,512]
        PSUM tiles copied to xnT (f32r) by Vector in 512-wide slabs.
      - Attention is ScalarE-exp-bound, so the PE stream is organized to keep
        exp back-to-back: S^T steps run ahead, and the O^T accumulation work
        of the PREVIOUS half-head is emitted as 8-link sub-chain bursts
        between S steps (the PE loses ~2x throughput on every S<->O stream
        transition, so transitions are kept to ~16/head instead of 64/head).
        exp output (E) is staged in an SBUF ring (~26 tiles) bridging the
        half-head production->consumption lag.
    """
    from collections import deque

    P = 128
    S, D = N, DIM
    ST = S // P          # 16 seq tiles
    DT = D // P          # 8 feat tiles
    NQ = S // 512        # 4 q chunks
    HD = HL * DH         # 256 local head cols
    QSL = S // LANES     # 512 output rows per core
    QT = QSL // P        # 4
    GROUPS = [list(range(N_CORES))]

    # v4: the whole QKV/attention pipeline runs bf16 (within the 2e-2
    # tolerance): bf16 weights DMA directly (no stage+cast), LDWEIGHTS gets
    # FWL (fp32 loads in 2 half passes), and the S matmul moves 1024 cols
    # per instruction instead of 512.
    MMF = F16

    x_in = nc.dram_tensor("x", [S, D], F32, kind="ExternalInput").ap()
    wq_in = nc.dram_tensor("wq", [D, HD], F16, kind="ExternalInput").ap()
    wk_in = nc.dram_tensor("wk", [D, HD], F16, kind="ExternalInput").ap()
    wv_in = nc.dram_tensor("wv", [D, HD], F16, kind="ExternalInput").ap()
    if has_bias:
        bq_in = nc.dram_tensor("bq", [HD], F32, kind="ExternalInput").ap()
        bk_in = nc.dram_tensor("bk", [HD], F32, kind="ExternalInput").ap()
        bv_in = nc.dram_tensor("bv", [HD], F32, kind="ExternalInput").ap()
    if use_a2a:
        # The 8-core AllToAll delivers both groups' head blocks; "sel" (per
        # core 1.0/0.0) drives a branch-free merge picking this core's group
        # so the output projection contracts only D (not 2D of half-zeros).
        wout_in = nc.dram_tensor("wout", [D, D], BF16,
                                 kind="ExternalInput").ap()
        sel_in = nc.dram_tensor("sel", [P], F32, kind="ExternalInput").ap()
        out_dram = nc.dram_tensor("out", [QSL, D], F32,
                                  kind="ExternalOutput").ap()
        a2a_in = [nc.dram_tensor(f"a2a_in{h}", [N_CORES, DH, QSL], BF16).ap()
                  for h in range(HL)]
        a2a_out = [nc.dram_tensor(f"a2a_out{h}", [N_CORES, DH, QSL], BF16).ap()
                   for h in range(HL)]
        KTO = DT         # out-proj contraction tiles
    else:
        # no-collective fallback: emit the local partial product over the
        # core's 4 heads for ALL rows; host sums the 4 partials per batch.
        wout_in = nc.dram_tensor("woutp", [HD, D], BF16,
                                 kind="ExternalInput").ap()
        out_dram = nc.dram_tensor("out", [S, D], F32,
                                  kind="ExternalOutput").ap()
        KTO = HD // P    # 2
    den_dram = [nc.dram_tensor(f"den{h}", [DH, S // DH], F32).ap()
                for h in range(HL)]
    denb_dram = [nc.dram_tensor(f"denb{h}", [DH, S // DH], BF16).ap()
                 for h in range(HL)]

    with tile.TileContext(nc) as tc:
        with (
            tc.tile_pool(name="const", bufs=1) as const,
            tc.tile_pool(name="big", bufs=1) as big,
        ):
            # ---- small constants ----
            eps_sb = const.tile([P, 1], F32)
            nc.vector.memset(eps_sb, EPS)
            # rewritten (to 0.0) after every LN Sqrt; used as the zero bias
            # of the early Exp activations so every Sqrt precedes every Exp
            # in the ScalarE stream (a sqrt<->exp flip costs a ~1.3us ACT
            # table reload)
            zero_gate = const.tile([P, 1], F32)
            nc.vector.memset(zero_gate, 0.0)
            ident = const.tile([P, P], F16)
            make_identity(nc, ident)
            if has_bias:
                ones_row = const.tile([1, 512], MMF)
                nc.vector.memset(ones_row, 1.0)

            # ---- activations that live through attention ----
            QT_sb = big.tile([P, HD // P, S], MMF)
            KT_sb = big.tile([P, HD // P, S], MMF)
            V_sb = big.tile([P, ST, HL, DH + 1], BF16)
            nc.vector.memset(V_sb[:, :, :, DH:DH + 1], 1.0)
            # exp outputs for the early (in-prefix) attention steps of
            # heads 0/1 (t 0-7, q chunks 0-1): [128, 512] quarters
            e_early = big.tile([P, 32, 512], BF16)
            e_map = {}
            e_done = set()
            EARLY = [(h, t, c) for h in (0, 1)
                     for t in range(8) for c in (0, 1)]
            if use_a2a:
                Ofull = big.tile([P, 2 * KTO, QSL], BF16)
                Om = big.tile([P, KTO, QSL], BF16)
                sel_sb = big.tile([P, 1], F32)
                nc.sync.dma_start(out=sel_sb,
                                  in_=sel_in.rearrange("(p o) -> p o", p=P))
            else:
                obf_all = big.tile([P, KTO, S], BF16)

            # ======== prefix: LN + transpose + projections (scoped) ========
            with (
                tc.tile_pool(name="xnp", bufs=1) as xnp,
                tc.tile_pool(name="wstage", bufs=1) as wstage,
                tc.tile_pool(name="xp", bufs=2) as xp,
                tc.tile_pool(name="xnbuf", bufs=2) as xnbuf,
                tc.tile_pool(name="stats", bufs=4) as stats,
            ):

                def load_weight(name, src):
                    # bf16 weights DMA straight into SBUF; the gpsimd queue
                    # keeps them off the sync queue that feeds x tiles.
                    w = xnp.tile([P, DT, HD], F16, tag=name, name=name)
                    nc.gpsimd.dma_start(
                        out=w, in_=src.rearrange("(o p) m -> p o m", p=P))
                    return w

                wk_sb = load_weight("wk", wk_in)
                wq_sb = load_weight("wq", wq_in)
                wv_sb = load_weight("wv", wv_in)

                bias_sb = {}
                if has_bias:
                    for name, src in (("bq", bq_in), ("bk", bk_in),
                                      ("bv", bv_in)):
                        bstage = wstage.tile([1, HD], F32, tag="bstage",
                                             name=f"bstage_{name}")
                        nc.gpsimd.dma_start(out=bstage, in_=src)
                        b = xnp.tile([1, HD], MMF, tag="bias", name=name)
                        nc.vector.tensor_copy(out=b, in_=bstage)
                        bias_sb[name] = b

                with (
                    tc.tile_pool(name="tp", bufs=2, space="PSUM") as tp,
                    tc.tile_pool(name="proj", bufs=2, space="PSUM") as proj,
                    tc.tile_pool(name="vproj", bufs=2,
                                 space="PSUM") as vproj,
                    tc.tile_pool(name="searly", bufs=2,
                                 space="PSUM") as searly,
                ):
                    def ln_tile(st, xnT):
                        x_t = xp.tile([P, D], F32, tag="x",
                                      name=f"x_{st}")
                        # spread x tiles over all three dispatch queues so
                        # many DMA engines stream concurrently (a single
                        # 512KB DMA takes ~20us on one queue engine); tile
                        # 0 is split 4 ways so the LN pipeline starts fast.
                        G = 4 if st < 2 else 1
                        engs = [nc.sync, nc.scalar]
                        for gi in range(G):
                            w0 = gi * (D // G)
                            engs[gi % 2].dma_start(
                                out=x_t[:, w0:w0 + D // G],
                                in_=x_in[st * P:(st + 1) * P,
                                         w0:w0 + D // G])
                        stt = stats.tile([P, 4, 6], F32, tag="stt")
                        GS = max(G, 2)
                        for gi in range(GS):
                            w0 = gi * (D // GS)
                            nc.vector.bn_stats(
                                out=stt[:, gi], in_=x_t[:, w0:w0 + D // GS])
                        mv = stats.tile([P, 2], F32, tag="mv")
                        nc.vector.bn_aggr(out=mv, in_=stt[:, 0:GS])
                        std = stats.tile([P, 1], F32, tag="std")
                        nc.scalar.activation(
                            out=std, in_=mv[:, 1:2],
                            func=mybir.ActivationFunctionType.Sqrt,
                            bias=eps_sb)
                        rstd = stats.tile([P, 1], F32, tag="rstd")
                        nc.vector.reciprocal(out=rstd, in_=std)
                        nc.vector.tensor_scalar_mul(
                            out=zero_gate, in0=std, scalar1=0.0)
                        xn_t = xnbuf.tile([P, D], F16, tag="xn",
                                          name=f"xn_{st}")
                        nc.vector.tensor_scalar(
                            out=xn_t, in0=x_t, scalar1=mv[:, 0:1],
                            scalar2=rstd,
                            op0=mybir.AluOpType.subtract,
                            op1=mybir.AluOpType.mult)
                        sti = st % 4
                        for g in range(2):
                            pt_ps = tp.tile([P, 512], F16, tag="tp")
                            for j in range(4):
                                ft = g * 4 + j
                                nc.tensor.transpose(
                                    pt_ps[:, j * P:(j + 1) * P],
                                    xn_t[:, ft * P:(ft + 1) * P], ident)
                            nc.vector.tensor_copy(
                                out=xnT[:, g * 4:(g + 1) * 4,
                                        sti * P:(sti + 1) * P],
                                in_=pt_ps.rearrange(
                                    "p (a b) -> p a b", a=4))

                    def qk_chain(w_sb, bname, dst, pt, nch, xnT):
                        ps = proj.tile([P, 512], F32, tag="proj")
                        for kt in range(DT):
                            nc.tensor.matmul(
                                ps, w_sb[:, kt, pt * P:(pt + 1) * P],
                                xnT[:, kt, :],
                                start=(kt == 0),
                                stop=(kt == DT - 1 and not has_bias))
                        if has_bias:
                            nc.tensor.matmul(
                                ps, bias_sb[bname][0:1, pt * P:(pt + 1) * P],
                                ones_row, start=False, stop=True)
                        nc.vector.tensor_copy(
                            out=dst[:, pt, nch * 512:(nch + 1) * 512],
                            in_=ps)

                    def early_step(i):
                        # S + exp for (h, t, c) that only needs K/Q chunks
                        # 0-1: overlaps ScalarE exp with the remaining
                        # prefix PE work.  The S matmul is unfloored (the
                        # scheduler places it mid-prefix on the PE), but the
                        # exp is floored past the whole modeled prefix so
                        # every LN Sqrt PRECEDES every Exp in the ScalarE
                        # stream -- otherwise the scheduler weaves them and
                        # each sqrt<->exp flip costs a ~1.3us ACT table
                        # load.
                        h, t, c = EARLY[i]
                        kb = (h * DH) % P
                        kpt = (h * DH) // P
                        s_ps = searly.tile([P, 512], F32, tag="se",
                                           name=f"se_{h}_{t}_{c}")
                        nc.tensor.matmul(
                            s_ps,
                            KT_sb[kb:kb + DH, kpt, t * P:(t + 1) * P],
                            QT_sb[kb:kb + DH, kpt, c * 512:(c + 1) * 512],
                            start=True, stop=True)
                        ei = e_early[:, i, :]
                        nc.scalar.activation(
                            out=ei, in_=s_ps,
                            func=mybir.ActivationFunctionType.Exp,
                            bias=zero_gate)
                        e_map[(h, t, c)] = ei
                        e_done.add((h, t, c))

                    for nch in range(NQ):
                        xnT = xnp.tile([P, DT, 512], MMF, tag="xnT",
                                       name=f"xnT_{nch}")
                        for sti in range(4):
                            ln_tile(nch * 4 + sti, xnT)
                        if nch == NQ - 1:
                            # all 16 LN Sqrt activations are now emitted, so
                            # the exp stream can start without thrashing the
                            # ACT table set; these overlap the last chunk's
                            # projection chains on the PE.
                            for i in range(len(EARLY)):
                                early_step(i)
                        for pt in range(HD // P):
                            qk_chain(wk_sb, "bk", KT_sb, pt, nch, xnT)
                        for pt in range(HD // P):
                            qk_chain(wq_sb, "bq", QT_sb, pt, nch, xnT)
                        for sti in range(4):
                            st = nch * 4 + sti
                            ps = vproj.tile([P, HD], F32, tag="vproj")
                            for kt in range(DT):
                                nc.tensor.matmul(
                                    ps, xnT[:, kt, sti * P:(sti + 1) * P],
                                    wv_sb[:, kt, :],
                                    start=(kt == 0),
                                    stop=(kt == DT - 1 and not has_bias))
                            if has_bias:
                                nc.tensor.matmul(
                                    ps, ones_row[0:1, 0:P], bias_sb["bv"],
                                    start=False, stop=True)
                            nc.vector.tensor_copy(
                                out=V_sb[:, st, :, 0:DH],
                                in_=ps.rearrange("p (h d) -> p h d", h=HL))


            # ======== attention (exp-bound, S-ahead / O-subchain bursts) ====
            with (
                tc.tile_pool(name="late", bufs=1) as late,
                tc.tile_pool(name="expp", bufs=26) as expp,
                tc.tile_pool(name="obfp", bufs=2) as obfp,
                tc.tile_pool(name="bcast", bufs=2) as bcast,
                tc.tile_pool(name="outp", bufs=2) as outp,
            ):
                # out-proj weights: DMA overlaps attention (allocated here
                # so the slot reuses SBUF freed by the prefix pools)
                wout_sb = late.tile([P, KTO, D], BF16, name="wout_sb")
                nc.sync.dma_start(out=wout_sb,
                                  in_=wout_in.rearrange("(o p) m -> p o m",
                                                        p=P))
                # two rotating reciprocal-broadcast buffers, zero-filled
                # once up front (stream_shuffle reads the whole tile)
                rec_bufs = [late.tile([DH, S], BF16, name=f"rec_buf{i}")
                            for i in range(2)]
                for rb in rec_bufs:
                    nc.vector.memset(rb, 0.0)

                def finish_head(h, o_ps):
                    # stage O_ext to SBUF at once: frees the 4 o_ps PSUM
                    # banks; normalize + AllToAll overlap the next head.
                    o_sb = bcast.tile([DH + 1, S], F32, tag="osum",
                                      name=f"o_sb_{h}")
                    for c in range(NQ):
                        nc.vector.tensor_copy(
                            out=o_sb[:, c * 512:(c + 1) * 512],
                            in_=o_ps[c])
                    # denominators live on partition 64 as a [1, S] row.
                    # Direct SBUF->SBUF partition-scatter DMA to a [64, 32]
                    # layout, recip there (single-partition recip is ~16us
                    # on DVE), cast bf16, gather back into partitions 0 and
                    # 32, then one DVE stream_shuffle broadcasts within
                    # each 32-partition quadrant.  All dispatches ride the
                    # sync queue: the gpsimd stream stalls at the
                    # Ofull-gather A2A waits, and the PE stream is
                    # untouched.
                    nc.sync.dma_start(out=den_dram[h].rearrange(
                        "j m -> (j m)"), in_=o_sb[DH:DH + 1, :])
                    dn = bcast.tile([DH, S // DH], F32, tag="dn",
                                    name=f"dn_{h}")
                    nc.sync.dma_start(out=dn, in_=den_dram[h])
                    nc.vector.reciprocal(out=dn, in_=dn)
                    dnr = bcast.tile([DH, S // DH], BF16, tag="dnr",
                                     name=f"dnr_{h}")
                    nc.vector.tensor_copy(out=dnr, in_=dn)
                    nc.sync.dma_start(out=denb_dram[h].rearrange(
                        "j m -> (j m)"), in_=dnr)
                    rec_b = rec_bufs[h % 2]
                    rbv = rec_b.rearrange("(a b) q -> a b q", a=2)
                    for a in range(2):
                        nc.sync.dma_start(
                            out=rbv[a:a + 1, 0:1, :],
                            in_=denb_dram[h].rearrange("j m -> (j m)"))
                    nc.vector.stream_shuffle(out=rec_b, in_=rec_b,
                                             mask=[0] * 32)
                    if use_a2a:
                        obf_h = obfp.tile([DH, S], BF16, tag="obf")
                    else:
                        inner = h * DH
                        obf_h = obf_all[inner % P:inner % P + DH,
                                        inner // P, :]
                    nc.vector.tensor_mul(
                        out=obf_h, in0=o_sb[0:DH, :], in1=rec_b)
                    if use_a2a:
                        # lane shard j duplicated into both groups' slots;
                        # 4 DMAs across two queues so the 512KB write is
                        # parallel across DMA engines.
                        for half in range(2):
                            for piece in range(2):
                                j0 = piece * 2
                                nc.sync.dma_start(
                                    out=a2a_in[h][half * LANES + j0:
                                                  half * LANES + j0 + 2]
                                    .rearrange("j p q -> p j q"),
                                    in_=obf_h[:, j0 * QSL:(j0 + 2) * QSL]
                                    .rearrange("p (j q) -> p j q", j=2))
                        nc.gpsimd.collective_compute(
                            "AllToAll", mybir.AluOpType.bypass,
                            replica_groups=GROUPS,
                            ins=[a2a_in[h][:]], outs=[a2a_out[h][:]])
                        # h-major gather: head h's blocks land at Ofull
                        # col-blocks g*8 + 2h + {0,1}; lanes (0,1)/(2,3)
                        # fill partitions 0-127 of each block.
                        for g in range(2):
                            nc.gpsimd.dma_start(
                                out=Ofull[:, g * KTO + 2 * h:
                                          g * KTO + 2 * h + 2, :],
                                in_=a2a_out[h][g * LANES:(g + 1) * LANES]
                                .rearrange("(lb lt) d q -> (lt d) lb q",
                                           lt=2))

                with (
                    tc.tile_pool(name="spsum", bufs=2, space="PSUM") as spsum,
                    tc.tile_pool(name="opsum", bufs=NQ, space="PSUM") as opsum,
                ):
                    pend = deque()
                    o_ps_by_head = {}
                    head_windows = {}

                    QL = ST // 2     # t-links per drained sub-chain

                    def check_window(h, t, c):
                        tlo = (t // QL) * QL
                        if all((h, t2, c) in e_done
                               for t2 in range(tlo, tlo + QL)):
                            pend.append((h, c, tlo))

                    def emit_subchain():
                        h2, c, tlo = pend.popleft()
                        if h2 not in o_ps_by_head:
                            o_ps_by_head[h2] = [
                                opsum.tile([DH + 1, 512], F32, tag="o",
                                           name=f"o_ps_{h2}_{cc}")
                                for cc in range(NQ)]
                        o_ps = o_ps_by_head[h2]
                        for t2 in range(tlo, tlo + QL):
                            nc.tensor.matmul(
                                o_ps[c], V_sb[:, t2, h2, :],
                                e_map[(h2, t2, c)],
                                start=(t2 == 0), stop=(t2 == ST - 1))
                        head_windows[h2] = head_windows.get(h2, 0) + 1
                        if head_windows[h2] == 2 * NQ:
                            finish_head(h2, o_ps_by_head.pop(h2))

                    # The TileScheduler reorders by modeled readiness and
                    # would round-robin S and O matmuls (each S<->O stream
                    # switch costs ~2x on the PE).  Modeled-time floors pin
                    # each S-step and each O-sub-chain into its own slot;
                    # they gate only the scheduler's simulated clock, no
                    # real waits are emitted.
                    ATT_MS = 1.0
                    STEP_MS = 0.003

                    def s_step(h, t, half, step):
                        kb = (h * DH) % P
                        kpt = (h * DH) // P
                        with tc.tile_wait_until(ATT_MS + step * STEP_MS,
                                                enable=False):
                            s_ps = spsum.tile(
                                [P, S // 2], F32, tag="s",
                                name=f"s_ps_{h}_{t}_{half}")
                            for cc in range(NQ // 2):
                                c = half * (NQ // 2) + cc
                                nc.tensor.matmul(
                                    s_ps[:, cc * 512:(cc + 1) * 512],
                                    KT_sb[kb:kb + DH, kpt,
                                          t * P:(t + 1) * P],
                                    QT_sb[kb:kb + DH, kpt,
                                          c * 512:(c + 1) * 512],
                                    start=True, stop=True)
                            e_t = expp.tile([P, S // 2], BF16,
                                            tag="e",
                                            name=f"e_t_{h}_{t}_{half}")
                            nc.scalar.activation(
                                out=e_t, in_=s_ps,
                                func=mybir.ActivationFunctionType.Exp)
                            for cc in range(NQ // 2):
                                c = half * (NQ // 2) + cc
                                e_map[(h, t, c)] = \
                                    e_t[:, cc * 512:(cc + 1) * 512]
                                e_done.add((h, t, c))
                        for cc in range(NQ // 2):
                            check_window(h, t, half * (NQ // 2) + cc)

                    # steps not already covered by the early (in-prefix)
                    # emission: heads 0/1 are missing half 1 of t0-7 and
                    # all of t8-15; heads 2/3 everything.
                    MAIN = []
                    for h in (0, 1):
                        MAIN += [(h, t, 1) for t in range(QL)]
                        MAIN += [(h, t, 0) for t in range(QL, ST)]
                        MAIN += [(h, t, 1) for t in range(QL, ST)]
                    for h in (2, 3):
                        for t in range(ST):
                            MAIN += [(h, t, 0), (h, t, 1)]
                    # windows fully produced by the early steps
                    for h in (0, 1):
                        for c in (0, 1):
                            check_window(h, 0, c)

                    step = 0
                    for (h, t, half) in MAIN:
                        s_step(h, t, half, step)
                        if step % 2 == 1 and pend:
                            emit_subchain()
                        step += 1
                    while pend:
                        emit_subchain()
                        step += 1

                # ---- merge + output projection ----
                # h-major Ofull layout: head h owns col-blocks {2h, 2h+1}
                # (group 0) and {2h+8, 2h+9} (group 1); Om[j] = sel ?
                # Ofull[j] : Ofull[j+KTO].  Phase 0 (heads 0-2, 48 links)
                # runs hidden under the h3 A2A wait; phase 1 (head 3, 16
                # links) is the only post-A2A tail.
                if use_a2a:
                    OM_MS = 2.0
                    with tc.tile_pool(name="oproj", bufs=8,
                                      space="PSUM") as oproj:
                        pss = {}
                        for phase, heads in enumerate(((0, 1, 2), (3,))):
                            ks = [j for hh in heads
                                  for j in (2 * hh, 2 * hh + 1)]
                            with tc.tile_wait_until(OM_MS + phase * 0.05):
                                for j in ks:
                                    mtmp = outp.tile([P, QSL], F32,
                                                     tag="mtmp")
                                    nc.vector.tensor_sub(
                                        out=mtmp,
                                        in0=Ofull[:, j, :],
                                        in1=Ofull[:, j + KTO, :])
                                    nc.vector.scalar_tensor_tensor(
                                        out=Om[:, j, :], in0=mtmp,
                                        scalar=sel_sb,
                                        in1=Ofull[:, j + KTO, :],
                                        op0=mybir.AluOpType.mult,
                                        op1=mybir.AluOpType.add)
                            with tc.tile_wait_until(OM_MS + 0.01
                                                    + phase * 0.05):
                                for qt in range(QT):
                                    for nch in range(D // 512):
                                        if phase == 0:
                                            pss[(qt, nch)] = oproj.tile(
                                                [P, 512], F32, tag="op",
                                                name=f"op_{qt}_{nch}")
                                        ps = pss[(qt, nch)]
                                        for i2, kt in enumerate(ks):
                                            nc.tensor.matmul(
                                                ps,
                                                Om[:, kt,
                                                   qt * P:(qt + 1) * P],
                                                wout_sb[:, kt,
                                                        nch * 512:
                                                        (nch + 1) * 512],
                                                start=(phase == 0 and
                                                       i2 == 0),
                                                stop=(phase == 1 and
                                                      i2 == len(ks) - 1))
                        for qt in range(QT):
                            ot = outp.tile([P, D], F32, tag="ot")
                            for nch in range(D // 512):
                                nc.vector.tensor_copy(
                                    out=ot[:, nch * 512:(nch + 1) * 512],
                                    in_=pss[(qt, nch)])
                            # alternate dispatch queues so the final 2MB
                            # write streams on more DMA engines (ScalarE is
                            # idle after the last exp)
                            eng = nc.sync if qt % 2 == 0 else nc.scalar
                            eng.dma_start(
                                out=out_dram[qt * P:(qt + 1) * P, :],
                                in_=ot)
                else:
                    with tc.tile_pool(name="oproj", bufs=4,
                                      space="PSUM") as oproj:
                        for qt in range(ST):
                            ot = outp.tile([P, D], F32, tag="ot")
                            for nch in range(D // 512):
                                ps = oproj.tile([P, 512], F32, tag="op")
                                for kt in range(KTO):
                                    nc.tensor.matmul(
                                        ps,
                                        obf_all[:, kt, qt * P:(qt + 1) * P],
                                        wout_sb[:, kt,
                                                nch * 512:(nch + 1) * 512],
                                        start=(kt == 0),
                                        stop=(kt == KTO - 1))
                                nc.vector.tensor_copy(
                                    out=ot[:, nch * 512:(nch + 1) * 512],
                                    in_=ps)
                            nc.sync.dma_start(
                                out=out_dram[qt * P:(qt + 1) * P, :],
                                in_=ot)

    return nc


# ---------------------------------------------------------------------------
# Host entry point
# ---------------------------------------------------------------------------

_CACHE = {}
USE_A2A = True
USE_F32R = True


def _get_program(has_bias=False):
    key = (USE_A2A, USE_F32R, has_bias)
    if key not in _CACHE:
        nc = bass.Bass("TRN2", target_bir_lowering=False, debug=False,
                       num_devices=N_CORES)
        build(nc, use_f32r=USE_F32R, use_a2a=USE_A2A, has_bias=has_bias)
        split_multi_waits(nc)
        _CACHE[key] = nc
    return _CACHE[key]


def _shard_inputs(x, gamma, beta, Wq, Wkv, Wout):
    import ml_dtypes
    x = np.asarray(x, dtype=np.float32)
    gamma = np.asarray(gamma, dtype=np.float32)
    beta = np.asarray(beta, dtype=np.float32)
    Wq = np.asarray(Wq, dtype=np.float32)
    Wkv = np.asarray(Wkv, dtype=np.float32)
    Wk, Wv = Wkv[:, :H * DH], Wkv[:, H * DH:]
    # LayerNorm affine folded into the projections:
    #   (xn*gamma + beta) @ W = xn @ (gamma[:,None]*W) + beta @ W
    Wq_g = gamma[:, None] * Wq
    Wk_g = gamma[:, None] * Wk
    Wv_g = gamma[:, None] * Wv
    has_bias = bool(np.any(beta))
    if has_bias:
        bq = beta @ Wq_g
        bk = beta @ Wk_g
        bv = beta @ Wv_g
    wout_f = np.asarray(Wout, np.float32)
    wout_bf = np.ascontiguousarray(wout_f).astype(ml_dtypes.bfloat16)
    # h-major inner layout for the A2A path: Om row h*256 + lane*64 + d
    # holds global head (lane*4 + h) -> permute Wout rows to match.
    wout_perm = np.ascontiguousarray(
        wout_f.reshape(LANES, HL, DH, DIM).transpose(1, 0, 2, 3)
        .reshape(DIM, DIM)).astype(ml_dtypes.bfloat16)
    in_maps = []
    for core in range(N_CORES):
        b = core // LANES
        lane = core % LANES
        cs = slice(lane * HL * DH, (lane + 1) * HL * DH)
        m = {
            "x": np.ascontiguousarray(x[b]),
            "wq": np.ascontiguousarray(Wq_g[:, cs]).astype(np.float16),
            "wk": np.ascontiguousarray(Wk_g[:, cs]).astype(np.float16),
            "wv": np.ascontiguousarray(Wv_g[:, cs]).astype(np.float16),
        }
        if has_bias:
            m["bq"] = np.ascontiguousarray(bq[cs])
            m["bk"] = np.ascontiguousarray(bk[cs])
            m["bv"] = np.ascontiguousarray(bv[cs])
        if USE_A2A:
            m["wout"] = wout_perm
            m["sel"] = np.full((128,), 1.0 if b == 0 else 0.0,
                               dtype=np.float32)
        else:
            m["woutp"] = np.ascontiguousarray(wout_bf[cs.start:cs.stop])
        in_maps.append(m)
    return in_maps


def _unshard_output(results):
    out = np.empty((B, N, DIM), dtype=np.float32)
    if USE_A2A:
        qsl = N // LANES
        for core in range(N_CORES):
            b = core // LANES
            lane = core % LANES
            out[b, lane * qsl:(lane + 1) * qsl, :] = results[core]["out"]
    else:
        for b in range(B):
            acc = results[b * LANES]["out"].astype(np.float32).copy()
            for lane in range(1, LANES):
                acc += results[b * LANES + lane]["out"]
            out[b] = acc
    return out


def kernel(x, gamma, beta, Wq, Wkv, Wout, trace=False):
    from concourse.bass_utils import run_bass_kernel_spmd
    has_bias = bool(np.any(np.asarray(beta, dtype=np.float32)))
    nc = _get_program(has_bias)
    in_maps = _shard_inputs(x, gamma, beta, Wq, Wkv, Wout)
    res = run_bass_kernel_spmd(nc, in_maps, list(range(N_CORES)), trace=trace)
    out = _unshard_output(res.results)
    if trace:
        kernel.last_exec_time_ns = res.exec_time_ns
        kernel.last_result = res
    return out

